# revision 1
# baseline (speedup 1.0000x reference)
"""GPT forward (8 layers, C=1024, T=1024, B=2, H=16, V=32000) on 8 trn2 cores.

Sharding: TP4 x DP2. Cores 0-3 handle batch 0, cores 4-7 batch 1.
Within a quad, core j owns heads 4j..4j+3, MLP hidden slice j*1024..,
and vocab slice j*8000.. of the LM head.

Device layout: the residual stream lives in SBUF transposed (xT: [C, T],
channels on partitions). All matmuls contract over the partition dim, so
weights (w[C,F] etc.) are natively the stationary lhsT operand and no
activation transposes are ever needed. LN stats (sums over C) are computed
on the PE with a ones[128,1] stationary vector. Softmax is max-free (logits
are provably tiny) with the denominator fused into the AV matmul via a ones
column appended to V. Matmuls run in bf16 with fp32 PSUM accumulation;
residual/softmax/LN math stays fp32.
"""

import numpy as np
import ml_dtypes

import concourse.bacc as bacc
import concourse.bass as bass
import concourse.tile as tile
import concourse.mybir as mybir
from concourse import bass_utils

f32 = mybir.dt.float32
bf16 = mybir.dt.bfloat16
AF = mybir.ActivationFunctionType
OP = mybir.AluOpType

B, T, C, L, H, F, V = 2, 1024, 1024, 8, 16, 4096, 32000
HD = C // H            # 64
TP = 4                 # tensor-parallel within a quad
HL = H // TP           # 4 local heads
QO = C // TP           # 256 local q/k/v width
FL = F // TP           # 1024 local mlp hidden
VL = V // TP           # 8000 local vocab
NCH = C // 128         # 8 channel chunks
NTC = T // 128         # 8 token chunks
GROUPS = [[0, 1, 2, 3], [4, 5, 6, 7]]
LN_EPS = 1e-5
SCALE = 1.0 / np.sqrt(HD)

_STATE = {}


def _build(collectives=True):
    nc = bacc.Bacc("TRN2", target_bir_lowering=False, debug=False,
                   enable_asserts=False, num_devices=8)

    x0T_d = nc.dram_tensor("x0t", [C, T], f32, kind="ExternalInput").ap()
    wqkv_d = nc.dram_tensor("wqkv", [L, C, 3 * QO], bf16, kind="ExternalInput").ap()
    w1_d = nc.dram_tensor("w1", [L, C, FL], bf16, kind="ExternalInput").ap()
    w2_d = nc.dram_tensor("w2", [L, FL, C], bf16, kind="ExternalInput").ap()
    hw_d = nc.dram_tensor("hw", [C, VL], bf16, kind="ExternalInput").ap()
    # per-partition constant columns (see host packing below)
    bqk_d = nc.dram_tensor("bqk", [128, L * 4], f32, kind="ExternalInput").ap()
    bvb_d = nc.dram_tensor("bvb", [L, 128, QO], f32, kind="ExternalInput").ap()
    b1_d = nc.dram_tensor("b1c", [128, L * 8], f32, kind="ExternalInput").ap()
    b2_d = nc.dram_tensor("b2c", [128, L * 8], f32, kind="ExternalInput").ap()
    ln1w_d = nc.dram_tensor("ln1w", [128, L * 8], f32, kind="ExternalInput").ap()
    ln1b_d = nc.dram_tensor("ln1b", [128, L * 8], f32, kind="ExternalInput").ap()
    ln2w_d = nc.dram_tensor("ln2w", [128, L * 8], f32, kind="ExternalInput").ap()
    ln2b_d = nc.dram_tensor("ln2b", [128, L * 8], f32, kind="ExternalInput").ap()
    lnfw_d = nc.dram_tensor("lnfw", [128, 8], f32, kind="ExternalInput").ap()
    lnfb_d = nc.dram_tensor("lnfb", [128, 8], f32, kind="ExternalInput").ap()
    mask_d = nc.dram_tensor("mask", [128, 128], bf16, kind="ExternalInput").ap()
    out_d = nc.dram_tensor("out", [T, VL], f32, kind="ExternalOutput").ap()

    with tile.TileContext(nc) as tc:
        _prog(nc, tc, x0T_d, wqkv_d, w1_d, w2_d, hw_d, bqk_d, bvb_d, b1_d,
              b2_d, ln1w_d, ln1b_d, ln2w_d, ln2b_d, lnfw_d, lnfb_d, mask_d,
              out_d, collectives)
    nc.compile()
    return nc


def _prog(nc, tc, x0T_d, wqkv_d, w1_d, w2_d, hw_d, bqk_d, bvb_d, b1_d, b2_d,
          ln1w_d, ln1b_d, ln2w_d, ln2b_d, lnfw_d, lnfb_d, mask_d, out_d,
          collectives=True):
    import contextlib
    ctx = contextlib.ExitStack()
    with ctx:
        const = ctx.enter_context(tc.tile_pool(name="const", bufs=1))
        xp = ctx.enter_context(tc.tile_pool(name="xres", bufs=NCH))
        hp = ctx.enter_context(tc.tile_pool(name="hln", bufs=NCH))
        qkp = ctx.enter_context(tc.tile_pool(name="qk", bufs=4))
        vp = ctx.enter_context(tc.tile_pool(name="vsb", bufs=32))
        sbf = ctx.enter_context(tc.tile_pool(name="scrbf", bufs=9))
        s32 = ctx.enter_context(tc.tile_pool(name="scr32", bufs=6))
        bc = ctx.enter_context(tc.tile_pool(name="bcast", bufs=2))
        yp = ctx.enter_context(tc.tile_pool(name="ysb", bufs=2))
        sm = ctx.enter_context(tc.tile_pool(name="small", bufs=4))
        wqp = ctx.enter_context(tc.tile_pool(name="wqkv", bufs=10))
        w1p = ctx.enter_context(tc.tile_pool(name="w1", bufs=9))
        w2p = ctx.enter_context(tc.tile_pool(name="w2", bufs=9))
        bvp = ctx.enter_context(tc.tile_pool(name="bvb", bufs=2))
        hwp = ctx.enter_context(tc.tile_pool(name="hwsb", bufs=16))
        psb = ctx.enter_context(tc.tile_pool(name="psbig", bufs=2, space="PSUM"))
        pss = ctx.enter_context(tc.tile_pool(name="pssm", bufs=2, space="PSUM"))
        dr = ctx.enter_context(tc.tile_pool(name="dram", bufs=2, space="DRAM"))

        ones = const.tile([128, 1], f32)
        nc.vector.memset(ones[:], 1.0)
        eps_t = const.tile([1, 1], f32, tag="eps")
        nc.vector.memset(eps_t[:], LN_EPS)
        mask = const.tile([128, 128], bf16)
        nc.sync.dma_start(mask[:], mask_d[:])
        cols = {}
        for nm, d, w in (("bqk", bqk_d, L * 4), ("b1", b1_d, L * 8),
                         ("b2", b2_d, L * 8), ("l1w", ln1w_d, L * 8),
                         ("l1b", ln1b_d, L * 8), ("l2w", ln2w_d, L * 8),
                         ("l2b", ln2b_d, L * 8), ("lfw", lnfw_d, 8),
                         ("lfb", lnfb_d, 8)):
            t = const.tile([128, w], f32, tag=f"c_{nm}")
            nc.sync.dma_start(t[:], d[:])
            cols[nm] = t

        # residual stream: 8 persistent fp32 tiles [128 ch, 1024 tok]
        xt = []
        for cc in range(NCH):
            t = xp.tile([128, T], f32)
            nc.sync.dma_start(t[:], x0T_d[cc * 128:(cc + 1) * 128, :])
            xt.append(t)

        def layernorm(wcol, bcol, coff):
            """xt -> list of 8 bf16 [128,T] normalized tiles."""
            ssum = pss.tile([1, T], f32, tag="pss")
            sqsum = pss.tile([1, T], f32, tag="pss")
            for cc in range(NCH):
                sq = s32.tile([128, T], f32, tag="s32")
                nc.scalar.activation(sq[:], xt[cc][:], AF.Square)
                for th in range(2):
                    sl = slice(th * 512, (th + 1) * 512)
                    nc.tensor.matmul(ssum[:, sl], ones[:], xt[cc][:, sl],
                                     start=(cc == 0), stop=(cc == NCH - 1))
                    nc.tensor.matmul(sqsum[:, sl], ones[:], sq[:, sl],
                                     start=(cc == 0), stop=(cc == NCH - 1))
            mu = sm.tile([1, T], f32, tag="sm")
            nc.vector.tensor_scalar_mul(mu[:], ssum[:], 1.0 / C)
            var = sm.tile([1, T], f32, tag="sm")
            # var = sqsum/C - mu^2  ->  (sqsum * 1/C) sub mu*mu
            mu2 = sm.tile([1, T], f32, tag="sm")
            nc.vector.tensor_mul(mu2[:], mu[:], mu[:])
            nc.vector.scalar_tensor_tensor(var[:], sqsum[:], 1.0 / C, mu2[:],
                                           op0=OP.mult, op1=OP.subtract)
            std = sm.tile([1, T], f32, tag="sm")
            nc.scalar.activation(std[:], var[:], AF.Sqrt, bias=eps_t[:])
            rstd = sm.tile([1, T], f32, tag="sm")
            nc.vector.reciprocal(rstd[:], std[:])
            nmr = sm.tile([1, T], f32, tag="sm")
            nc.vector.scalar_tensor_tensor(nmr[:], mu[:], -1.0, rstd[:],
                                           op0=OP.mult, op1=OP.mult)
            rstd_b = bc.tile([128, T], f32, tag="bc")
            nc.gpsimd.partition_broadcast(rstd_b[:], rstd[:])
            nmr_b = bc.tile([128, T], f32, tag="bc")
            nc.gpsimd.partition_broadcast(nmr_b[:], nmr[:])
            out = []
            for cc in range(NCH):
                t1 = s32.tile([128, T], f32, tag="s32")
                nc.vector.tensor_mul(t1[:], xt[cc][:], rstd_b[:])
                nc.vector.tensor_add(t1[:], t1[:], nmr_b[:])
                h = hp.tile([128, T], bf16)
                co = coff + cc
                nc.scalar.activation(h[:], t1[:], AF.Identity,
                                     scale=wcol[:, co:co + 1],
                                     bias=bcol[:, co:co + 1])
                out.append(h)
            return out

        for l in range(L):
            wq_t = []
            for cc in range(NCH):
                t = wqp.tile([128, 3 * QO], bf16)
                nc.sync.dma_start(t[:], wqkv_d[l, cc * 128:(cc + 1) * 128, :])
                wq_t.append(t)

            h1 = layernorm(cols["l1w"], cols["l1b"], l * 8)

            # q,k in transposed [qo, T] layout (2 chunks each)
            qk_t = []
            for oc in range(4):
                p = psb.tile([128, T], f32, tag="psb")
                for th in range(2):
                    sl = slice(th * 512, (th + 1) * 512)
                    for cc in range(NCH):
                        nc.tensor.matmul(p[:, sl],
                                         wq_t[cc][:, oc * 128:(oc + 1) * 128],
                                         h1[cc][:, sl],
                                         start=(cc == 0), stop=(cc == NCH - 1))
                dst = qkp.tile([128, T], bf16)
                nc.vector.tensor_scalar_add(dst[:], p[:],
                                            cols["bqk"][:, l * 4 + oc:l * 4 + oc + 1])
                qk_t.append(dst)

            # v in normal [tok, vo] layout, split per head with a ones column
            bvt = bvp.tile([128, QO], f32)
            nc.sync.dma_start(bvt[:], bvb_d[l, :, :])
            v_t = [[None] * HL for _ in range(NTC)]
            for tcc in range(NTC):
                pv = pss.tile([128, QO], f32, tag="pss")
                for cc in range(NCH):
                    nc.tensor.matmul(pv[:], h1[cc][:, tcc * 128:(tcc + 1) * 128],
                                     wq_t[cc][:, 2 * QO:3 * QO],
                                     start=(cc == 0), stop=(cc == NCH - 1))
                for hh in range(HL):
                    vt = vp.tile([128, HD + 1], bf16)
                    nc.vector.memset(vt[:, HD:HD + 1], 1.0)
                    nc.vector.tensor_add(vt[:, 0:HD], pv[:, hh * HD:(hh + 1) * HD],
                                         bvt[:, hh * HD:(hh + 1) * HD])
                    v_t[tcc][hh] = vt

            # attention per local head; y accumulated into 2 fp32 tiles [128, T]
            y_sb = [yp.tile([128, T], f32, tag="y", name=f"ysb{i}") for i in range(2)]
            for hh in range(HL):
                qi, ro = hh // 2, (hh % 2) * 64
                att = []
                for si in range(NTC):
                    pa = psb.tile([128, T], f32, tag="psb")
                    lhs = qk_t[2 + qi][ro:ro + 64, si * 128:(si + 1) * 128]
                    for th in range(si // 4, 2):
                        sl = slice(th * 512, (th + 1) * 512)
                        nc.tensor.matmul(pa[:, sl], lhs,
                                         qk_t[qi][ro:ro + 64, sl],
                                         start=True, stop=True)
                    ab = sbf.tile([128, T], bf16, tag="sbf")
                    sc = si * 128
                    if si % 4:
                        nc.vector.memset(ab[:, (si // 4) * 512:sc], 0.0)
                    nc.scalar.activation(ab[:, sc:T], pa[:, sc:T], AF.Exp,
                                         scale=float(SCALE))
                    nc.vector.tensor_mul(ab[:, sc:sc + 128], ab[:, sc:sc + 128],
                                         mask[:])
                    att.append(ab)
                py = pss.tile([HD + 1, T], f32, tag="pss")
                for th in range(2):
                    last = 3 if th == 0 else 7
                    sl = slice(th * 512, (th + 1) * 512)
                    for si in range(last + 1):
                        nc.tensor.matmul(py[:, sl], v_t[si][hh][:],
                                         att[si][:, sl],
                                         start=(si == 0), stop=(si == last))
                den_r = sm.tile([1, T], f32, tag="sm")
                nc.vector.reciprocal(den_r[:], py[HD:HD + 1, :])
                den_b = bc.tile([64, T], f32, tag="bc")
                nc.gpsimd.partition_broadcast(den_b[:], den_r[:])
                nc.vector.tensor_mul(y_sb[hh // 2][(hh % 2) * 64:(hh % 2) * 64 + 64, :],
                                     py[0:HD, :], den_b[:])

            # AllGather y within quad -> full yT, add to residual
            g_in = dr.tile([QO, T], f32, tag="gin")
            for i in range(2):
                nc.sync.dma_start(g_in[i * 128:(i + 1) * 128, :], y_sb[i][:])
            g_out = dr.tile([C, T], f32, tag="gout")
            if collectives is True:
                nc.gpsimd.collective_compute("AllGather", OP.bypass,
                                             replica_groups=GROUPS,
                                             ins=[g_in.opt()], outs=[g_out.opt()])
            elif collectives == "local":
                for q in range(TP):
                    nc.sync.dma_start(g_out[q * QO:(q + 1) * QO, :], g_in[:])
            for cc in range(NCH):
                yt = s32.tile([128, T], f32, tag="s32")
                nc.sync.dma_start(yt[:], g_out[cc * 128:(cc + 1) * 128, :]
                                  if collectives != "skip"
                                  else g_in[(cc % 2) * 128:(cc % 2) * 128 + 128, :])
                nc.vector.tensor_add(xt[cc][:], xt[cc][:], yt[:])

            # MLP
            w1_t, w2_t = [], []
            for cc in range(NCH):
                t = w1p.tile([128, FL], bf16)
                nc.sync.dma_start(t[:], w1_d[l, cc * 128:(cc + 1) * 128, :])
                w1_t.append(t)
                t = w2p.tile([128, C], bf16)
                nc.sync.dma_start(t[:], w2_d[l, cc * 128:(cc + 1) * 128, :])
                w2_t.append(t)

            h2 = layernorm(cols["l2w"], cols["l2b"], l * 8)
            a_t = []
            for fc in range(NCH):
                pm = psb.tile([128, T], f32, tag="psb")
                for th in range(2):
                    sl = slice(th * 512, (th + 1) * 512)
                    for cc in range(NCH):
                        nc.tensor.matmul(pm[:, sl],
                                         w1_t[cc][:, fc * 128:(fc + 1) * 128],
                                         h2[cc][:, sl],
                                         start=(cc == 0), stop=(cc == NCH - 1))
                ga = sbf.tile([128, T], bf16, tag="sbf")
                nc.scalar.activation(ga[:], pm[:], AF.Gelu,
                                     bias=cols["b1"][:, l * 8 + fc:l * 8 + fc + 1])
                a_t.append(ga)

            r_in = dr.tile([C, T], f32, tag="rin")
            for cc in range(NCH):
                pm2 = psb.tile([128, T], f32, tag="psb")
                for th in range(2):
                    sl = slice(th * 512, (th + 1) * 512)
                    for fc in range(NCH):
                        nc.tensor.matmul(pm2[:, sl],
                                         w2_t[fc][:, cc * 128:(cc + 1) * 128],
                                         a_t[fc][:, sl],
                                         start=(fc == 0), stop=(fc == NCH - 1))
                mo = s32.tile([128, T], f32, tag="s32")
                nc.vector.tensor_copy(mo[:], pm2[:])
                nc.sync.dma_start(r_in[cc * 128:(cc + 1) * 128, :], mo[:])
            r_out = dr.tile([C, T], f32, tag="rout")
            if collectives is True:
                nc.gpsimd.collective_compute("AllReduce", OP.add,
                                             replica_groups=GROUPS,
                                             ins=[r_in.opt()], outs=[r_out.opt()])
            elif collectives == "local":
                nc.sync.dma_start(r_out[:], r_in[:])
            for cc in range(NCH):
                rt = s32.tile([128, T], f32, tag="s32")
                nc.sync.dma_start(rt[:], r_out[cc * 128:(cc + 1) * 128, :]
                                  if collectives != "skip"
                                  else r_in[cc * 128:(cc + 1) * 128, :])
                nc.vector.scalar_tensor_tensor(
                    xt[cc][:], rt[:], cols["b2"][:, l * 8 + cc:l * 8 + cc + 1],
                    xt[cc][:], op0=OP.add, op1=OP.add)

        # final LN + LM head (normal orientation: out[tok, vocab])
        hf = layernorm(cols["lfw"], cols["lfb"], 0)
        NVB = (VL + 511) // 512
        for vb in range(NVB):
            vn = min(512, VL - vb * 512)
            rhs_t = []
            for cc in range(NCH):
                wt = hwp.tile([128, 512], bf16)
                nc.sync.dma_start(wt[:, 0:vn],
                                  hw_d[cc * 128:(cc + 1) * 128,
                                       vb * 512:vb * 512 + vn])
                rhs_t.append(wt)
            for tcc in range(NTC):
                ph = psb.tile([128, 512], f32, tag="psb")
                for cc in range(NCH):
                    nc.tensor.matmul(ph[:, 0:vn],
                                     hf[cc][:, tcc * 128:(tcc + 1) * 128],
                                     rhs_t[cc][:, 0:vn],
                                     start=(cc == 0), stop=(cc == NCH - 1))
                so = s32.tile([128, T], f32, tag="s32")
                if tcc % 2:
                    nc.vector.tensor_copy(so[:, 0:vn], ph[:, 0:vn])
                else:
                    nc.scalar.activation(so[:, 0:vn], ph[:, 0:vn], AF.Copy)
                nc.sync.dma_start(out_d[tcc * 128:(tcc + 1) * 128,
                                        vb * 512:vb * 512 + vn],
                                  so[:, 0:vn])


def _prep_inputs(idx, tok_emb, pos_emb, ln1_w, ln1_b, wq, bq, wk, bk, wv, bv,
                 ln2_w, ln2_b, w1, b1, w2, b2, lnf_w, lnf_b, head_w):
    bf = ml_dtypes.bfloat16

    def cols128(a):  # [L, C] -> [128, L*8] per-partition column packing
        a = np.ascontiguousarray(a, np.float32)
        Lx = a.shape[0]
        return a.reshape(Lx, NCH, 128).transpose(2, 0, 1).reshape(128, Lx * NCH)

    mask = np.zeros((128, 128), np.float32)
    p, t = np.meshgrid(np.arange(128), np.arange(128), indexing="ij")
    mask[p <= t] = 1.0
    in_maps = []
    shard_cache = {}
    x0s = [np.ascontiguousarray(
        (tok_emb[np.asarray(idx[g], np.int64)] + pos_emb[0]).T, np.float32)
        for g in range(B)]
    for c in range(8):
        g, j = c // 4, c % 4
        if j in shard_cache:
            m = dict(shard_cache[j])
            m["x0t"] = x0s[g]
            in_maps.append(m)
            continue
        x0 = tok_emb[np.asarray(idx[g], np.int64)] + pos_emb[0]
        m = {
            "x0t": np.ascontiguousarray(x0.T, np.float32),
            "wqkv": np.ascontiguousarray(np.concatenate(
                [wq[:, :, j * QO:(j + 1) * QO], wk[:, :, j * QO:(j + 1) * QO],
                 wv[:, :, j * QO:(j + 1) * QO]], axis=2)).astype(bf),
            "w1": np.ascontiguousarray(w1[:, :, j * FL:(j + 1) * FL]).astype(bf),
            "w2": np.ascontiguousarray(w2[:, j * FL:(j + 1) * FL, :]).astype(bf),
            "hw": np.ascontiguousarray(head_w[:, j * VL:(j + 1) * VL]).astype(bf),
            "bqk": np.ascontiguousarray(np.stack(
                [bq[:, j * QO:(j + 1) * QO].reshape(L, 2, 128),
                 bk[:, j * QO:(j + 1) * QO].reshape(L, 2, 128)],
                axis=1).reshape(L * 4, 128).T, np.float32),
            "bvb": np.ascontiguousarray(np.broadcast_to(
                bv[:, None, j * QO:(j + 1) * QO], (L, 128, QO)), np.float32),
            "b1c": cols128(b1[:, j * FL:(j + 1) * FL]),
            "b2c": cols128(b2),
            "ln1w": cols128(ln1_w), "ln1b": cols128(ln1_b),
            "ln2w": cols128(ln2_w), "ln2b": cols128(ln2_b),
            "lnfw": cols128(lnf_w[None]), "lnfb": cols128(lnf_b[None]),
            "mask": mask.astype(bf),
        }
        m["x0t"] = x0s[g]
        shard_cache[j] = m
        in_maps.append(m)
    return in_maps


def kernel(**inputs):
    if "nc" not in _STATE:
        _STATE["nc"] = _build()
    nc = _STATE["nc"]
    in_maps = _prep_inputs(**{k: np.asarray(v) for k, v in inputs.items()})
    res = bass_utils.run_bass_kernel_spmd(nc, in_maps, core_ids=list(range(8)))
    outs = res.results
    full = np.empty((B, T, V), np.float32)
    for c in range(8):
        g, j = c // 4, c % 4
        full[g, :, j * VL:(j + 1) * VL] = outs[c]["out"]
    return full



# revision 7
# speedup vs baseline: 1.4921x; 1.4921x over previous
"""GPT forward (8 layers, C=1024, T=1024, B=2, H=16, V=32000) on 8 trn2 cores.

Sharding: TP4 x DP2. Cores 0-3 handle batch 0, cores 4-7 batch 1.
Within a quad, core j owns heads 4j..4j+3, MLP hidden slice j*1024..,
and vocab slice j*8000.. of the LM head.

v2 design notes:
- Residual stream lives in SBUF as fp16, transposed ([C, T] with channels
  on partitions), split into two T/2 token halves that are software-
  pipelined through the whole network so collectives overlap compute.
- LayerNorm is folded into the weights on the host (W <- W * ln_w), and
  all biases (ln_b contributions + layer biases) ride the existing
  PSUM->SBUF casts as per-partition bias columns, so normalization on
  device is only: stats matmuls (ones-vector trick, fp16 at 1 cyc/row),
  a tiny per-token scalar chain, and x2 = x*rstd + (-mu*rstd) per chunk.
- Attention is max-free softmax (exp then divide by the ones-column
  denominator folded into the AV matmul), with causality exploited at
  128-column granularity (ragged score matmuls, triangular AV chains).
- Collectives (y AllGather, MLP AllReduce) are fp16, per half, and are
  covered by the other half's compute in the pipeline.
"""

import numpy as np
import ml_dtypes

import concourse.bacc as bacc
import concourse.bass as bass
import concourse.tile as tile
import concourse.mybir as mybir
from concourse import bass_utils

f32 = mybir.dt.float32
f16 = mybir.dt.float16
bf16 = mybir.dt.bfloat16
AF = mybir.ActivationFunctionType
OP = mybir.AluOpType

B, T, C, L, H, F, V = 2, 1024, 1024, 8, 16, 4096, 32000
HD = C // H            # 64
TP = 4                 # tensor-parallel within a quad
HL = H // TP           # 4 local heads
QO = C // TP           # 256 local q/k/v width
FL = F // TP           # 1024 local mlp hidden
VL = V // TP           # 8000 local vocab
NCH = C // 128         # 8 channel chunks
TH = T // 2            # 512 tokens per half
GROUPS = [[0, 1, 2, 3], [4, 5, 6, 7]]
LN_EPS = 1e-5
SCALE = 1.0 / np.sqrt(HD)

_STATE = {}


def _build(collectives=True):
    nc = bacc.Bacc("TRN2", target_bir_lowering=False, debug=False,
                   enable_asserts=False, num_devices=8)

    x0T_d = nc.dram_tensor("x0t", [C, T], f16, kind="ExternalInput").ap()
    wqkv_d = nc.dram_tensor("wqkv", [L, C, 3 * QO], f16, kind="ExternalInput").ap()
    w1_d = nc.dram_tensor("w1", [L, C, FL], f16, kind="ExternalInput").ap()
    w2_d = nc.dram_tensor("w2", [L, FL, C], f16, kind="ExternalInput").ap()
    hw_d = nc.dram_tensor("hw", [C, VL], f16, kind="ExternalInput").ap()
    # per-partition bias columns (all layer biases + folded-LN bias terms)
    bqk_d = nc.dram_tensor("bqk", [128, L * 4], f32, kind="ExternalInput").ap()
    by_d = nc.dram_tensor("byc", [128, L * 2], f32, kind="ExternalInput").ap()
    b1_d = nc.dram_tensor("b1c", [128, L * 8], f32, kind="ExternalInput").ap()
    b2_d = nc.dram_tensor("b2c", [128, L * 8], f32, kind="ExternalInput").ap()
    bf_d = nc.dram_tensor("bfc", [128, 8], f32, kind="ExternalInput").ap()
    mask_d = nc.dram_tensor("mask", [128, 128], bf16, kind="ExternalInput").ap()
    out_d = nc.dram_tensor("out", [T, VL], f16, kind="ExternalOutput").ap()

    with tile.TileContext(nc) as tc:
        _prog(nc, tc, x0T_d, wqkv_d, w1_d, w2_d, hw_d, bqk_d, by_d, b1_d,
              b2_d, bf_d, mask_d, out_d, collectives)
    nc.compile()
    return nc


def _prog(nc, tc, x0T_d, wqkv_d, w1_d, w2_d, hw_d, bqk_d, by_d, b1_d, b2_d,
          bf_d, mask_d, out_d, collectives=True):
    import contextlib
    ctx = contextlib.ExitStack()
    with ctx:
        const = ctx.enter_context(tc.tile_pool(name="const", bufs=1))
        xp = ctx.enter_context(tc.tile_pool(name="xres", bufs=1))
        x2p = ctx.enter_context(tc.tile_pool(name="x2", bufs=19))
        t1p = ctx.enter_context(tc.tile_pool(name="t1", bufs=3))
        sqp = ctx.enter_context(tc.tile_pool(name="sq", bufs=3))
        qkp = ctx.enter_context(tc.tile_pool(name="qk", bufs=5))
        vp = ctx.enter_context(tc.tile_pool(name="vsb", bufs=10))
        abp = ctx.enter_context(tc.tile_pool(name="ab", bufs=9))
        ap_ = ctx.enter_context(tc.tile_pool(name="act", bufs=10))
        yrp = ctx.enter_context(tc.tile_pool(name="yraw", bufs=3))
        ybp = ctx.enter_context(tc.tile_pool(name="ybf", bufs=4))
        bcp = ctx.enter_context(tc.tile_pool(name="bcast", bufs=5))
        dbp = ctx.enter_context(tc.tile_pool(name="denb", bufs=2))
        mop = ctx.enter_context(tc.tile_pool(name="mo", bufs=6))
        rbp = ctx.enter_context(tc.tile_pool(name="rb", bufs=6))
        smp = ctx.enter_context(tc.tile_pool(name="small", bufs=8))
        wqp = ctx.enter_context(tc.tile_pool(name="wqkv", bufs=14))
        w1p = ctx.enter_context(tc.tile_pool(name="w1", bufs=9))
        w2p = ctx.enter_context(tc.tile_pool(name="w2", bufs=9))
        hwp = ctx.enter_context(tc.tile_pool(name="hwsb", bufs=10))
        outp = ctx.enter_context(tc.tile_pool(name="outs", bufs=3))
        psA = ctx.enter_context(tc.tile_pool(name="psA", bufs=2, space="PSUM"))
        psB = ctx.enter_context(tc.tile_pool(name="psB", bufs=2, space="PSUM"))
        psC = ctx.enter_context(tc.tile_pool(name="psC", bufs=2, space="PSUM"))
        psS = ctx.enter_context(tc.tile_pool(name="psS", bufs=2, space="PSUM"))
        dr = ctx.enter_context(tc.tile_pool(name="dram", bufs=2, space="DRAM"))

        ones16 = const.tile([128, 1], f16)
        nc.vector.memset(ones16[:], 1.0)
        eps_t = const.tile([1, 1], f32, tag="eps")
        nc.vector.memset(eps_t[:], LN_EPS)
        mask = const.tile([128, 128], bf16)
        nc.sync.dma_start(mask[:], mask_d[:])
        bqk_c = const.tile([128, L * 4], f32, tag="bqk")
        nc.sync.dma_start(bqk_c[:], bqk_d[:])
        by_c = const.tile([128, L * 2], f32, tag="byc")
        nc.sync.dma_start(by_c[:], by_d[:])
        b1_c = const.tile([128, L * 8], f32, tag="b1c")
        nc.sync.dma_start(b1_c[:], b1_d[:])
        b2_c = const.tile([128, L * 8], f32, tag="b2c")
        nc.sync.dma_start(b2_c[:], b2_d[:])
        bf_c = const.tile([128, 8], f32, tag="bfc")
        nc.sync.dma_start(bf_c[:], bf_d[:])

        # residual stream: per (half, chunk) fp16 [128, 512], persistent
        xt = [[None] * NCH for _ in range(2)]
        for h in range(2):
            for cc in range(NCH):
                t = xp.tile([128, TH], f16, tag=f"x{h}_{cc}")
                nc.sync.dma_start(t[:], x0T_d[cc * 128:(cc + 1) * 128,
                                               h * TH:(h + 1) * TH])
                xt[h][cc] = t

        def ln_stats(h):
            """stats + per-token chain -> (rstd_b, nmr_b) fp16 [128, TH]."""
            ssum = psS.tile([1, TH], f32, tag="st")
            sqsum = psS.tile([1, TH], f32, tag="st")
            for cc in range(NCH):
                sq = sqp.tile([128, TH], f16, tag="sq")
                nc.vector.tensor_mul(sq[:], xt[h][cc][:], xt[h][cc][:])
                nc.tensor.matmul(ssum[:], ones16[:], xt[h][cc][:],
                                 start=(cc == 0), stop=(cc == NCH - 1))
                nc.tensor.matmul(sqsum[:], ones16[:], sq[:],
                                 start=(cc == 0), stop=(cc == NCH - 1))
            mu = smp.tile([1, TH], f32, tag="sm")
            nc.vector.tensor_scalar_mul(mu[:], ssum[:], 1.0 / C)
            mu2 = smp.tile([1, TH], f32, tag="sm")
            nc.vector.tensor_mul(mu2[:], mu[:], mu[:])
            var = smp.tile([1, TH], f32, tag="sm")
            nc.vector.scalar_tensor_tensor(var[:], sqsum[:], 1.0 / C, mu2[:],
                                           op0=OP.mult, op1=OP.subtract)
            lnv = smp.tile([1, TH], f32, tag="sm")
            nc.scalar.activation(lnv[:], var[:], AF.Ln, bias=eps_t[:])
            rstd = smp.tile([1, TH], f32, tag="sm")
            nc.scalar.activation(rstd[:], lnv[:], AF.Exp, scale=-0.5)
            nmr = smp.tile([1, TH], f32, tag="sm")
            nc.vector.scalar_tensor_tensor(nmr[:], mu[:], -1.0, rstd[:],
                                           op0=OP.mult, op1=OP.mult)
            r16 = smp.tile([1, TH], f16, tag="sm16")
            nc.vector.tensor_copy(r16[:], rstd[:])
            n16 = smp.tile([1, TH], f16, tag="sm16")
            nc.vector.tensor_copy(n16[:], nmr[:])
            rstd_b = bcp.tile([128, TH], f16, tag="bc")
            nc.gpsimd.partition_broadcast(rstd_b[:], r16[:])
            nmr_b = bcp.tile([128, TH], f16, tag="bc")
            nc.gpsimd.partition_broadcast(nmr_b[:], n16[:])
            return rstd_b, nmr_b

        def ln_norm(h, rstd_b, nmr_b, cc):
            """x2 = x*rstd + nmr for one chunk -> fp16 tile."""
            t1 = t1p.tile([128, TH], f16, tag="t1")
            nc.vector.tensor_mul(t1[:], xt[h][cc][:], rstd_b[:])
            x2 = x2p.tile([128, TH], f16, tag="x2")
            nc.vector.tensor_add(x2[:], t1[:], nmr_b[:])
            return x2

        r_out_prev = [None, None]   # AR output dram tiles from prev layer

        for l in range(L):
            # weight loads for this layer (slot-gated by the pools)
            wq_t = []
            for cc in range(NCH):
                t = wqp.tile([128, 3 * QO], f16, tag="wq")
                nc.sync.dma_start(t[:], wqkv_d[l, cc * 128:(cc + 1) * 128, :])
                wq_t.append(t)
            w1_t, w2_t = [], []
            for cc in range(NCH):
                t = w1p.tile([128, FL], f16, tag="w1")
                nc.sync.dma_start(t[:], w1_d[l, cc * 128:(cc + 1) * 128, :])
                w1_t.append(t)
                t = w2p.tile([128, C], f16, tag="w2")
                nc.sync.dma_start(t[:], w2_d[l, cc * 128:(cc + 1) * 128, :])
                w2_t.append(t)

            x2h = [None, None]      # x2 chunks per half (LN1)
            qh = [[None] * 2, [None] * 2]
            kh = [[None] * 2, [None] * 2]
            vh = [[None] * 4, [None] * 4]   # v tiles [128, 4, 65] per (h, tv)
            ab = {}                 # (h, hh, si) -> exp'd score tile
            ybf = [[None] * 2, [None] * 2]
            gio = [None, None]

            for h in range(2):
                # --- residual update from prev layer's AllReduce ---
                if l > 0:
                    for cc in range(NCH):
                        rt = rbp.tile([128, TH], f16, tag="rb")
                        nc.sync.dma_start(
                            rt[:], r_out_prev[h][cc * 128:(cc + 1) * 128, :])
                        nc.gpsimd.tensor_add(xt[h][cc][:], xt[h][cc][:], rt[:])
                # --- LN1 + QKV for this half ---
                rstd_b, nmr_b = ln_stats(h)
                x2h[h] = [ln_norm(h, rstd_b, nmr_b, cc) for cc in range(NCH)]
                for oc in range(4):
                    p = psA.tile([128, TH], f32, tag="pmm")
                    for cc in range(NCH):
                        nc.tensor.matmul(p[:], wq_t[cc][:, oc * 128:(oc + 1) * 128],
                                         x2h[h][cc][:],
                                         start=(cc == 0), stop=(cc == NCH - 1))
                    dst = qkp.tile([128, TH], f16,
                                   tag=("qh" if oc < 2 else "kh"))
                    nc.scalar.activation(dst[:], p[:], AF.Identity,
                                         bias=bqk_c[:, l * 4 + oc:l * 4 + oc + 1])
                    if oc < 2:
                        qh[h][oc] = dst
                    else:
                        kh[h][oc - 2] = dst
                for tv in range(4):
                    pv = psA.tile([128, 4, HD], f32, tag="pmm")
                    for cc in range(NCH):
                        nc.tensor.matmul(pv[:, :, :],
                                         x2h[h][cc][:, tv * 128:(tv + 1) * 128],
                                         wq_t[cc][:, 2 * QO:3 * QO],
                                         start=(cc == 0), stop=(cc == NCH - 1))
                    vt = vp.tile([128, 4, HD + 1], bf16, tag="v")
                    nc.vector.memset(vt[:, :, HD:HD + 1], 1.0)
                    nc.scalar.activation(vt[:, :, 0:HD], pv[:, :, :], AF.Identity)
                    vh[h][tv] = vt

            for h in range(2):
                # --- attention for query half h ---
                for hh in range(HL):
                    qi, ro = hh // 2, (hh % 2) * 64
                    for si in range(4 * h + 4):
                        c0 = max(h * TH, si * 128)          # global col start
                        w = h * TH + TH - c0
                        pa = psB.tile([128, TH], f32, tag="pa")
                        nc.tensor.matmul(
                            pa[:, 0:w],
                            kh[si // 4][qi][ro:ro + 64,
                                            (si % 4) * 128:(si % 4) * 128 + 128],
                            qh[h][qi][ro:ro + 64, c0 - h * TH:c0 - h * TH + w],
                            start=True, stop=True)
                        abt = abp.tile([128, TH], bf16, tag="ab")
                        nc.scalar.activation(abt[:, 0:w], pa[:, 0:w], AF.Exp,
                                             scale=float(SCALE))
                        if c0 == si * 128:                   # diagonal block
                            nc.vector.tensor_mul(abt[:, 0:128], abt[:, 0:128],
                                                 mask[:])
                        ab[(h, hh, si)] = (abt, c0)
                    py = psC.tile([HD + 1, TH], f32, tag="py")
                    for tcl in range(4):
                        tcg = 4 * h + tcl
                        for si in range(tcg + 1):
                            abt, c0 = ab[(h, hh, si)]
                            nc.tensor.matmul(
                                py[:, tcl * 128:(tcl + 1) * 128],
                                vh[si // 4][si % 4][:, hh:hh + 1, :],
                                abt[:, tcg * 128 - c0:tcg * 128 - c0 + 128],
                                start=(si == 0), stop=(si == tcg))
                    den = smp.tile([1, TH], f32, tag="den", bufs=3)
                    nc.vector.reciprocal(den[:], py[HD:HD + 1, :])
                    den_b = dbp.tile([64, TH], f32, tag="db")
                    nc.gpsimd.partition_broadcast(den_b[:], den[:])
                    yraw = yrp.tile([64, TH], f16, tag="yr")
                    nc.vector.tensor_mul(yraw[:], py[0:HD, :], den_b[:])
                    i = hh // 2
                    if ybf[h][i] is None:
                        ybf[h][i] = ybp.tile([128, TH], f16, tag="yb",
                                             name=f"yb{l}_{h}_{i}")
                    ro2 = (hh % 2) * 64
                    nc.scalar.activation(
                        ybf[h][i][ro2:ro2 + 64, :], yraw[:], AF.Identity,
                        bias=by_c[ro2:ro2 + 64, l * 2 + i:l * 2 + i + 1])
                # --- y AllGather for half h ---
                g_in = dr.tile([QO, TH], f16, tag=f"gi{h}")
                for i in range(2):
                    nc.sync.dma_start(g_in[i * 128:(i + 1) * 128, :],
                                      ybf[h][i][:])
                g_out = dr.tile([C, TH], f16, tag=f"go{h}")
                if collectives is True:
                    nc.gpsimd.collective_compute(
                        "AllGather", OP.bypass, replica_groups=GROUPS,
                        ins=[g_in.opt()], outs=[g_out.opt()])
                else:
                    for q in range(TP):
                        nc.sync.dma_start(g_out[q * QO:(q + 1) * QO, :],
                                          g_in[:])
                gio[h] = g_out

            for h in range(2):
                # --- residual += y ---
                for cc in range(NCH):
                    yt = rbp.tile([128, TH], f16, tag="rb")
                    nc.sync.dma_start(yt[:],
                                      gio[h][cc * 128:(cc + 1) * 128, :])
                    nc.gpsimd.tensor_add(xt[h][cc][:], xt[h][cc][:], yt[:])
                # --- LN2 + MLP for this half ---
                rstd_b, nmr_b = ln_stats(h)
                x2m = [ln_norm(h, rstd_b, nmr_b, cc) for cc in range(NCH)]
                a_t = []
                for fc in range(NCH):
                    pm = psA.tile([128, TH], f32, tag="pmm")
                    for cc in range(NCH):
                        nc.tensor.matmul(pm[:],
                                         w1_t[cc][:, fc * 128:(fc + 1) * 128],
                                         x2m[cc][:],
                                         start=(cc == 0), stop=(cc == NCH - 1))
                    ga = ap_.tile([128, TH], f16, tag="a")
                    nc.scalar.activation(ga[:], pm[:], AF.Gelu,
                                         bias=b1_c[:, l * 8 + fc:l * 8 + fc + 1])
                    a_t.append(ga)
                r_in = dr.tile([C, TH], f16, tag=f"ri{h}")
                for cc in range(NCH):
                    pm2 = psA.tile([128, TH], f32, tag="pmm")
                    for fc in range(NCH):
                        nc.tensor.matmul(pm2[:],
                                         w2_t[fc][:, cc * 128:(cc + 1) * 128],
                                         a_t[fc][:],
                                         start=(fc == 0), stop=(fc == NCH - 1))
                    mo = mop.tile([128, TH], f16, tag="mo")
                    nc.vector.tensor_scalar_add(
                        mo[:], pm2[:], b2_c[:, l * 8 + cc:l * 8 + cc + 1])
                    nc.sync.dma_start(r_in[cc * 128:(cc + 1) * 128, :], mo[:])
                r_out = dr.tile([C, TH], f16, tag=f"ro{h}")
                if collectives is True:
                    nc.gpsimd.collective_compute(
                        "AllReduce", OP.add, replica_groups=GROUPS,
                        ins=[r_in.opt()], outs=[r_out.opt()])
                else:
                    nc.sync.dma_start(r_out[:], r_in[:])
                r_out_prev[h] = r_out

        # ---- final LN + LM head ----
        hf = [[None] * NCH, [None] * NCH]
        for h in range(2):
            for cc in range(NCH):
                rt = rbp.tile([128, TH], f16, tag="rb")
                nc.sync.dma_start(rt[:],
                                  r_out_prev[h][cc * 128:(cc + 1) * 128, :])
                nc.gpsimd.tensor_add(xt[h][cc][:], xt[h][cc][:], rt[:])
            rstd_b, nmr_b = ln_stats(h)
            for cc in range(NCH):
                t1 = t1p.tile([128, TH], f16, tag="t1")
                nc.vector.tensor_mul(t1[:], xt[h][cc][:], rstd_b[:])
                t2 = t1p.tile([128, TH], f16, tag="t1b")
                nc.vector.tensor_add(t2[:], t1[:], nmr_b[:])
                x2 = x2p.tile([128, TH], f16, tag="x2")
                nc.scalar.activation(x2[:], t2[:], AF.Identity,
                                     bias=bf_c[:, cc:cc + 1])
                hf[h][cc] = x2

        NVB = (VL + 511) // 512
        for vb in range(NVB):
            vn = min(512, VL - vb * 512)
            rhs_t = []
            for cc in range(NCH):
                wt = hwp.tile([128, 512], f16, tag="hw")
                nc.sync.dma_start(wt[:, 0:vn],
                                  hw_d[cc * 128:(cc + 1) * 128,
                                       vb * 512:vb * 512 + vn])
                rhs_t.append(wt)
            for tcc in range(8):
                h, tl = tcc // 4, tcc % 4
                ph = psA.tile([128, 512], f32, tag="pmm")
                for cc in range(NCH):
                    nc.tensor.matmul(ph[:, 0:vn],
                                     hf[h][cc][:, tl * 128:(tl + 1) * 128],
                                     rhs_t[cc][:, 0:vn],
                                     start=(cc == 0), stop=(cc == NCH - 1))
                so = outp.tile([128, 512], f16, tag="so")
                if tcc % 2:
                    nc.vector.tensor_copy(so[:, 0:vn], ph[:, 0:vn])
                else:
                    nc.scalar.activation(so[:, 0:vn], ph[:, 0:vn], AF.Copy)
                nc.sync.dma_start(out_d[tcc * 128:(tcc + 1) * 128,
                                        vb * 512:vb * 512 + vn],
                                  so[:, 0:vn])


def _prep_inputs(idx, tok_emb, pos_emb, ln1_w, ln1_b, wq, bq, wk, bk, wv, bv,
                 ln2_w, ln2_b, w1, b1, w2, b2, lnf_w, lnf_b, head_w):
    fh = np.float16

    mask = np.zeros((128, 128), np.float32)
    p, t = np.meshgrid(np.arange(128), np.arange(128), indexing="ij")
    mask[p <= t] = 1.0
    mask = mask.astype(ml_dtypes.bfloat16)

    x0s = []
    for g in range(B):
        x0 = tok_emb[np.asarray(idx[g], np.int64)] + pos_emb[0]
        x0s.append(np.ascontiguousarray(x0.T, np.float32).astype(fh))

    ln1w = np.asarray(ln1_w, np.float32)
    ln1b = np.asarray(ln1_b, np.float32)
    ln2w = np.asarray(ln2_w, np.float32)
    ln2b = np.asarray(ln2_b, np.float32)

    in_maps = []
    shard_cache = {}
    for c in range(8):
        g, j = c // 4, c % 4
        if j in shard_cache:
            m = dict(shard_cache[j])
            m["x0t"] = x0s[g]
            in_maps.append(m)
            continue
        sl = slice(j * QO, (j + 1) * QO)
        wq_j = np.asarray(wq[:, :, sl], np.float32)
        wk_j = np.asarray(wk[:, :, sl], np.float32)
        wv_j = np.asarray(wv[:, :, sl], np.float32)
        w1_j = np.asarray(w1[:, :, j * FL:(j + 1) * FL], np.float32)
        w2_j = np.asarray(w2[:, j * FL:(j + 1) * FL, :], np.float32)
        hw_j = np.asarray(head_w[:, j * VL:(j + 1) * VL], np.float32)

        # fold LN scale into the consuming weights
        wqkv = np.concatenate([wq_j, wk_j, wv_j], axis=2) * ln1w[:, :, None]
        w1f = w1_j * ln2w[:, :, None]
        hwf = hw_j * np.asarray(lnf_w, np.float32)[:, None]

        # bias columns: layer bias + W^T ln_b (the folded-LN bias term)
        # bqk: [128, L*4], col l*4 + oc covers q0,q1,k0,k1
        bqk = np.zeros((L, 4, 128), np.float32)
        byc = np.zeros((L, 2, 128), np.float32)
        for l in range(L):
            q_eff = bq[l, sl] + wq_j[l].T @ ln1b[l]
            k_eff = bk[l, sl] + wk_j[l].T @ ln1b[l]
            v_eff = bv[l, sl] + wv_j[l].T @ ln1b[l]
            bqk[l, 0] = q_eff[0:128]
            bqk[l, 1] = q_eff[128:256]
            bqk[l, 2] = k_eff[0:128]
            bqk[l, 3] = k_eff[128:256]
            byc[l, 0] = v_eff[0:128]
            byc[l, 1] = v_eff[128:256]
        bqk = np.ascontiguousarray(bqk.reshape(L * 4, 128).T)
        byc = np.ascontiguousarray(byc.reshape(L * 2, 128).T)

        b1c = np.zeros((L, NCH, 128), np.float32)
        for l in range(L):
            f_eff = b1[l, j * FL:(j + 1) * FL] + w1_j[l].T @ ln2b[l]
            b1c[l] = f_eff.reshape(NCH, 128)
        b1c = np.ascontiguousarray(b1c.reshape(L * 8, 128).T)

        b2c = np.ascontiguousarray(
            (np.asarray(b2, np.float32) / TP).reshape(L, NCH, 128)
            .reshape(L * 8, 128).T)

        lw = np.asarray(lnf_w, np.float32)
        lb = np.asarray(lnf_b, np.float32)
        bfc = np.where(np.abs(lw) > 1e-12, lb / np.where(lw == 0, 1, lw), 0.0)
        bfc = np.ascontiguousarray(bfc.reshape(NCH, 128).T.astype(np.float32))

        m = {
            "x0t": x0s[g],
            "wqkv": np.ascontiguousarray(wqkv).astype(fh),
            "w1": np.ascontiguousarray(w1f).astype(fh),
            "w2": np.ascontiguousarray(w2_j).astype(fh),
            "hw": np.ascontiguousarray(hwf).astype(fh),
            "bqk": bqk, "byc": byc, "b1c": b1c, "b2c": b2c, "bfc": bfc,
            "mask": mask,
        }
        shard_cache[j] = m
        in_maps.append(m)
    return in_maps


def kernel(**inputs):
    if "nc" not in _STATE:
        _STATE["nc"] = _build()
    nc = _STATE["nc"]
    in_maps = _prep_inputs(**{k: np.asarray(v) for k, v in inputs.items()})
    res = bass_utils.run_bass_kernel_spmd(nc, in_maps, core_ids=list(range(8)))
    outs = res.results
    full = np.empty((B, T, V), np.float32)
    for c in range(8):
        g, j = c // 4, c % 4
        full[g, :, j * VL:(j + 1) * VL] = np.asarray(outs[c]["out"],
                                                     np.float32)
    return full


# revision 43
# speedup vs baseline: 1.5189x; 1.0180x over previous
"""GPT forward (8 layers, C=1024, T=1024, B=2, H=16, V=32000) on 8 trn2 cores.

Sharding: TP4 x DP2. Cores 0-3 handle batch 0, cores 4-7 batch 1.
Within a quad, core j owns heads 4j..4j+3, MLP hidden slice j*1024..,
and vocab slice j*8000.. of the LM head.

v2 design notes:
- Residual stream lives in SBUF as fp16, transposed ([C, T] with channels
  on partitions), split into two T/2 token halves that are software-
  pipelined through the whole network so collectives overlap compute.
- LayerNorm is folded into the weights on the host (W <- W * ln_w), and
  all biases (ln_b contributions + layer biases) ride the existing
  PSUM->SBUF casts as per-partition bias columns, so normalization on
  device is only: stats matmuls (ones-vector trick, fp16 at 1 cyc/row),
  a tiny per-token scalar chain, and x2 = x*rstd + (-mu*rstd) per chunk.
- Attention is max-free softmax (exp then divide by the ones-column
  denominator folded into the AV matmul), with causality exploited at
  128-column granularity (ragged score matmuls, triangular AV chains).
- Collectives (y AllGather, MLP AllReduce) are fp16, per half, and are
  covered by the other half's compute in the pipeline.
"""

import numpy as np
import ml_dtypes

import concourse.bacc as bacc
import concourse.bass as bass
import concourse.tile as tile
import concourse.mybir as mybir
from concourse import bass_utils

f32 = mybir.dt.float32
f16 = mybir.dt.float16
bf16 = mybir.dt.bfloat16
AF = mybir.ActivationFunctionType
OP = mybir.AluOpType

B, T, C, L, H, F, V = 2, 1024, 1024, 8, 16, 4096, 32000
HD = C // H            # 64
TP = 4                 # tensor-parallel within a quad
HL = H // TP           # 4 local heads
QO = C // TP           # 256 local q/k/v width
FL = F // TP           # 1024 local mlp hidden
VL = V // TP           # 8000 local vocab
NCH = C // 128         # 8 channel chunks
TH = T // 2            # 512 tokens per half
GROUPS = [[0, 1, 2, 3], [4, 5, 6, 7]]
LN_EPS = 1e-5
SCALE = 1.0 / np.sqrt(HD)

_STATE = {}
_PHASE_LOG = []


def _steer_act_tables(arch):
    """The act-table-load pass greedily picks the first table set containing
    a function; `natural_log` (ln-only) shadows `natural_log_exp_and_others`,
    forcing a reload on every Ln->Exp pair in the LN chain. Empty the ln-only
    set (names and indices stay valid) so ln and exp share one table."""
    import concourse.hw_specs as hw_specs
    try:
        tables = hw_specs.get_activation_tables(arch)
        for name in ("natural_log",):
            if name in tables:
                tables[name].clear()
    except Exception:
        pass


def _build(collectives=True):
    nc = bacc.Bacc("TRN2", target_bir_lowering=False, debug=False,
                   enable_asserts=False, num_devices=8)
    _steer_act_tables(nc.m.arch)

    x0T_d = nc.dram_tensor("x0t", [C, T], f16, kind="ExternalInput").ap()
    wqkv_d = nc.dram_tensor("wqkv", [L, C, 3 * QO], f16, kind="ExternalInput").ap()
    w1_d = nc.dram_tensor("w1", [L, C, FL], f16, kind="ExternalInput").ap()
    w2_d = nc.dram_tensor("w2", [L, FL, C], f16, kind="ExternalInput").ap()
    hw_d = nc.dram_tensor("hw", [C, VL], f16, kind="ExternalInput").ap()
    # per-partition bias columns (all layer biases + folded-LN bias terms)
    bqk_d = nc.dram_tensor("bqk", [128, L * 4], f32, kind="ExternalInput").ap()
    by_d = nc.dram_tensor("byc", [128, L * 2], f32, kind="ExternalInput").ap()
    b1_d = nc.dram_tensor("b1c", [128, L * 8], f32, kind="ExternalInput").ap()
    b2_d = nc.dram_tensor("b2c", [128, L * 8], f32, kind="ExternalInput").ap()
    bf_d = nc.dram_tensor("bfc", [128, 8], f32, kind="ExternalInput").ap()
    mask_d = nc.dram_tensor("mask", [128, 128], bf16, kind="ExternalInput").ap()
    out_d = nc.dram_tensor("out", [T, VL], f16, kind="ExternalOutput").ap()

    with tile.TileContext(nc) as tc:
        _prog(nc, tc, x0T_d, wqkv_d, w1_d, w2_d, hw_d, bqk_d, by_d, b1_d,
              b2_d, bf_d, mask_d, out_d, collectives)
    nc.compile()
    return nc


def _prog(nc, tc, x0T_d, wqkv_d, w1_d, w2_d, hw_d, bqk_d, by_d, b1_d, b2_d,
          bf_d, mask_d, out_d, collectives=True):
    def mark(label):
        _PHASE_LOG.append((int(nc.next_id()), label))
    import contextlib
    ctx = contextlib.ExitStack()
    with ctx:
        const = ctx.enter_context(tc.tile_pool(name="const", bufs=1))
        xp = ctx.enter_context(tc.tile_pool(name="xres", bufs=1))
        x2p = ctx.enter_context(tc.tile_pool(name="x2", bufs=17))
        t1p = ctx.enter_context(tc.tile_pool(name="t1", bufs=2))
        sqp = ctx.enter_context(tc.tile_pool(name="sq", bufs=2))
        qkp = ctx.enter_context(tc.tile_pool(name="qk", bufs=4))
        vp = ctx.enter_context(tc.tile_pool(name="vsb", bufs=9))
        abp = ctx.enter_context(tc.tile_pool(name="ab", bufs=18))
        ap_ = ctx.enter_context(tc.tile_pool(name="act", bufs=10))
        yrp = ctx.enter_context(tc.tile_pool(name="yraw", bufs=2))
        ybp = ctx.enter_context(tc.tile_pool(name="ybf", bufs=4))
        bcp = ctx.enter_context(tc.tile_pool(name="bcast", bufs=4))
        dbp = ctx.enter_context(tc.tile_pool(name="denb", bufs=2))
        mop = ctx.enter_context(tc.tile_pool(name="mo", bufs=3))
        rbp = ctx.enter_context(tc.tile_pool(name="rb", bufs=3))
        smp = ctx.enter_context(tc.tile_pool(name="small", bufs=8))
        wqp = ctx.enter_context(tc.tile_pool(name="wqkv", bufs=14))
        w1p = ctx.enter_context(tc.tile_pool(name="w1", bufs=8))
        w2p = ctx.enter_context(tc.tile_pool(name="w2", bufs=8))
        hwp = ctx.enter_context(tc.tile_pool(name="hwsb", bufs=11))
        outp = ctx.enter_context(tc.tile_pool(name="outs", bufs=3))
        psA = ctx.enter_context(tc.tile_pool(name="psA", bufs=2, space="PSUM"))
        psB = ctx.enter_context(tc.tile_pool(name="psB", bufs=2, space="PSUM"))
        psC = ctx.enter_context(tc.tile_pool(name="psC", bufs=2, space="PSUM"))
        psS = ctx.enter_context(tc.tile_pool(name="psS", bufs=2, space="PSUM"))
        dr = ctx.enter_context(tc.tile_pool(name="dram", bufs=2, space="DRAM"))

        ones16 = const.tile([128, 1], f16)
        nc.vector.memset(ones16[:], 1.0)
        eps_t = const.tile([1, 1], f32, tag="eps")
        nc.vector.memset(eps_t[:], LN_EPS)
        mask = const.tile([128, 128], bf16)
        nc.sync.dma_start(mask[:], mask_d[:])
        bqk_c = const.tile([128, L * 4], f32, tag="bqk")
        nc.sync.dma_start(bqk_c[:], bqk_d[:])
        by_c = const.tile([128, L * 2], f32, tag="byc")
        nc.sync.dma_start(by_c[:], by_d[:])
        b1_c = const.tile([128, L * 8], f32, tag="b1c")
        nc.sync.dma_start(b1_c[:], b1_d[:])
        b2_c = const.tile([128, L * 8], f32, tag="b2c")
        nc.sync.dma_start(b2_c[:], b2_d[:])
        bf_c = const.tile([128, 8], f32, tag="bfc")
        nc.sync.dma_start(bf_c[:], bf_d[:])

        # residual stream: per (half, chunk) fp16 [128, 512], persistent
        xt = [[None] * NCH for _ in range(2)]
        for h in range(2):
            for cc in range(NCH):
                t = xp.tile([128, TH], f16, tag=f"x{h}_{cc}")
                nc.sync.dma_start(t[:], x0T_d[cc * 128:(cc + 1) * 128,
                                               h * TH:(h + 1) * TH])
                xt[h][cc] = t

        def ln_stats(h):
            """stats + per-token chain -> (rstd_b, nmr_b) fp16 [128, TH]."""
            ssum = psS.tile([1, TH], f32, tag="st")
            sqsum = psS.tile([1, TH], f32, tag="st")
            for cc in range(NCH):
                sq = sqp.tile([128, TH], f16, tag="sq")
                nc.vector.tensor_mul(sq[:], xt[h][cc][:], xt[h][cc][:])
                nc.tensor.matmul(ssum[:], ones16[:], xt[h][cc][:],
                                 start=(cc == 0), stop=(cc == NCH - 1))
                nc.tensor.matmul(sqsum[:], ones16[:], sq[:],
                                 start=(cc == 0), stop=(cc == NCH - 1))
            mu = smp.tile([1, TH], f32, tag="sm")
            nc.vector.tensor_scalar_mul(mu[:], ssum[:], 1.0 / C)
            mu2 = smp.tile([1, TH], f32, tag="sm")
            nc.vector.tensor_mul(mu2[:], mu[:], mu[:])
            var = smp.tile([1, TH], f32, tag="sm")
            nc.vector.scalar_tensor_tensor(var[:], sqsum[:], 1.0 / C, mu2[:],
                                           op0=OP.mult, op1=OP.subtract)
            lnv = smp.tile([1, TH], f32, tag="sm")
            nc.scalar.activation(lnv[:], var[:], AF.Ln, bias=eps_t[:])
            r16 = smp.tile([1, TH], f16, tag="sm16")
            nc.scalar.activation(r16[:], lnv[:], AF.Exp, scale=-0.5)
            n16 = smp.tile([1, TH], f16, tag="sm16")
            nc.vector.scalar_tensor_tensor(n16[:], mu[:], -1.0, r16[:],
                                           op0=OP.mult, op1=OP.mult)
            rstd_b = bcp.tile([128, TH], f16, tag="bc")
            nc.gpsimd.partition_broadcast(rstd_b[:], r16[:])
            nmr_b = bcp.tile([128, TH], f16, tag="bc")
            nc.gpsimd.partition_broadcast(nmr_b[:], n16[:])
            return rstd_b, nmr_b

        def ln_norm(h, rstd_b, nmr_b, cc):
            """x2 = x*rstd + nmr for one chunk -> fp16 tile."""
            t1 = t1p.tile([128, TH], f16, tag="t1")
            nc.vector.tensor_mul(t1[:], xt[h][cc][:], rstd_b[:])
            x2 = x2p.tile([128, TH], f16, tag="x2")
            nc.vector.tensor_add(x2[:], t1[:], nmr_b[:])
            return x2

        r_out_prev = [None, None]   # AR output dram tiles per half
        LW = {}                     # current layer's weight tiles
        S = {}                      # rolling attention state

        def emit_resm(h):
            for g in range(2):
                rt = rbp.tile([128, 4, TH], f16, tag="rb")
                nc.sync.dma_start(
                    rt[:, :, :],
                    r_out_prev[h][g * 4:(g + 1) * 4, :, :].transpose([1, 0, 2]))
                for k in range(4):
                    cc = g * 4 + k
                    nc.vector.tensor_add(xt[h][cc][:], xt[h][cc][:],
                                         rt[:, k, :])

        def emit_ln(h):
            rstd_b, nmr_b = ln_stats(h)
            return [ln_norm(h, rstd_b, nmr_b, cc) for cc in range(NCH)]

        def emit_qkv(l, h, x2):
            wq_t = LW["wq"]
            qk = []
            for oc in range(4):
                p = psA.tile([128, TH], f32, tag="pmm")
                for cc in range(NCH):
                    nc.tensor.matmul(p[:], wq_t[cc][:, oc * 128:(oc + 1) * 128],
                                     x2[cc][:],
                                     start=(cc == 0), stop=(cc == NCH - 1))
                dst = qkp.tile([128, TH], f16, tag=("qh" if oc < 2 else "kh"))
                nc.vector.tensor_scalar_add(
                    dst[:], p[:], bqk_c[:, l * 4 + oc:l * 4 + oc + 1])
                qk.append(dst)
            vts = []
            for tv in range(4):
                pv = psA.tile([128, 4, HD], f32, tag="pmm")
                for cc in range(NCH):
                    nc.tensor.matmul(pv[:, :, :],
                                     x2[cc][:, tv * 128:(tv + 1) * 128],
                                     wq_t[cc][:, 2 * QO:3 * QO],
                                     start=(cc == 0), stop=(cc == NCH - 1))
                vt = vp.tile([128, 4, HD + 1], bf16, tag="v")
                nc.vector.memset(vt[:, :, HD:HD + 1], 1.0)
                nc.vector.tensor_copy(vt[:, :, 0:HD], pv[:, :, :])
                vts.append(vt)
            S[("q", h)] = qk[:2]
            S[("k", h)] = qk[2:]
            S[("v", h)] = vts

        def emit_scores_head(h, hh):
            qi, ro = hh // 2, (hh % 2) * 64
            for si in range(4 * h + 4):
                c0 = max(h * TH, si * 128)
                w = h * TH + TH - c0
                pa = psB.tile([128, TH], f32, tag="pa")
                nc.tensor.matmul(
                    pa[:, 0:w],
                    S[("k", si // 4)][qi][ro:ro + 64,
                                          (si % 4) * 128:(si % 4) * 128 + 128],
                    S[("q", h)][qi][ro:ro + 64, c0 - h * TH:c0 - h * TH + w],
                    start=True, stop=True)
                abt = abp.tile([128, TH], bf16, tag="ab")
                nc.scalar.activation(abt[:, 0:w], pa[:, 0:w], AF.Exp,
                                     scale=float(SCALE))
                if c0 == si * 128:
                    nc.vector.tensor_mul(abt[:, 0:128], abt[:, 0:128], mask[:])
                S[("ab", h, hh, si)] = (abt, c0)

        def emit_av_head(l, h, hh, ybf):
            py = psC.tile([HD + 1, TH], f32, tag="py")
            for tcl in range(4):
                tcg = 4 * h + tcl
                for si in range(tcg + 1):
                    abt, c0 = S[("ab", h, hh, si)]
                    nc.tensor.matmul(
                        py[:, tcl * 128:(tcl + 1) * 128],
                        S[("v", si // 4)][si % 4][:, hh:hh + 1, :],
                        abt[:, tcg * 128 - c0:tcg * 128 - c0 + 128],
                        start=(si == 0), stop=(si == tcg))
            den = smp.tile([1, TH], f32, tag="den", bufs=3)
            nc.vector.reciprocal(den[:], py[HD:HD + 1, :])
            den_b = dbp.tile([64, TH], f32, tag="db")
            nc.gpsimd.partition_broadcast(den_b[:], den[:])
            yraw = yrp.tile([64, TH], f16, tag="yr")
            nc.vector.tensor_mul(yraw[:], py[0:HD, :], den_b[:])
            i = hh // 2
            if ybf[i] is None:
                ybf[i] = ybp.tile([128, TH], f16, tag="yb",
                                  name=f"yb{l}_{h}_{i}")
            ro2 = (hh % 2) * 64
            nc.scalar.activation(ybf[i][ro2:ro2 + 64, :], yraw[:], AF.Identity,
                                 bias=by_c[ro2:ro2 + 64, l * 2 + i:l * 2 + i + 1])

        def emit_ag(h, ybf):
            g_in = dr.tile([QO, TH], f16, tag=f"gi{h}")
            for i in range(2):
                nc.sync.dma_start(g_in[i * 128:(i + 1) * 128, :], ybf[i][:])
            g_out = dr.tile([NCH, 128, TH], f16, tag=f"go{h}")
            if collectives is True:
                nc.gpsimd.collective_compute(
                    "AllGather", OP.bypass, replica_groups=GROUPS,
                    ins=[g_in.opt()], outs=[g_out.opt()])
            else:
                for q in range(TP):
                    nc.sync.dma_start(g_out[q * 2:(q + 1) * 2, :, :],
                                      g_in[:])
            S[("go", h)] = g_out

        def emit_resy(h):
            for g in range(2):
                yt = rbp.tile([128, 4, TH], f16, tag="rb")
                nc.sync.dma_start(
                    yt[:, :, :],
                    S[("go", h)][g * 4:(g + 1) * 4, :, :].transpose([1, 0, 2]))
                for k in range(4):
                    cc = g * 4 + k
                    nc.vector.tensor_add(xt[h][cc][:], xt[h][cc][:],
                                         yt[:, k, :])

        def emit_mlp1(l, h, x2):
            a_t = []
            for fc in range(NCH):
                pm = psA.tile([128, TH], f32, tag="pmm")
                for cc in range(NCH):
                    nc.tensor.matmul(pm[:],
                                     LW["w1"][cc][:, fc * 128:(fc + 1) * 128],
                                     x2[cc][:],
                                     start=(cc == 0), stop=(cc == NCH - 1))
                ga = ap_.tile([128, TH], f16, tag="a")
                nc.scalar.activation(ga[:], pm[:], AF.Gelu,
                                     bias=b1_c[:, l * 8 + fc:l * 8 + fc + 1])
                a_t.append(ga)
            return a_t

        def emit_mlp2_ar(l, h, a_t):
            r_in = dr.tile([NCH, 128, TH], f16, tag=f"ri{h}")
            for cc in range(NCH):
                pm2 = psA.tile([128, TH], f32, tag="pmm")
                for fc in range(NCH):
                    nc.tensor.matmul(pm2[:],
                                     LW["w2"][fc][:, cc * 128:(cc + 1) * 128],
                                     a_t[fc][:],
                                     start=(fc == 0), stop=(fc == NCH - 1))
                mo = mop.tile([128, TH], f16, tag="mo")
                nc.scalar.activation(mo[:], pm2[:], AF.Identity,
                                     bias=b2_c[:, l * 8 + cc:l * 8 + cc + 1])
                nc.sync.dma_start(r_in[cc, :, :], mo[:])
            r_out = dr.tile([NCH, 128, TH], f16, tag=f"ro{h}")
            for g in range(2):
                if collectives is True:
                    nc.gpsimd.collective_compute(
                        "AllReduce", OP.add, replica_groups=GROUPS,
                        ins=[r_in[g * 4:(g + 1) * 4, :, :].opt()],
                        outs=[r_out[g * 4:(g + 1) * 4, :, :].opt()])
                else:
                    nc.sync.dma_start(r_out[g * 4:(g + 1) * 4, :, :],
                                      r_in[g * 4:(g + 1) * 4, :, :])
            r_out_prev[h] = r_out

        def emit_fin(h):
            rstd_b, nmr_b = ln_stats(h)
            out = []
            for cc in range(NCH):
                t1 = t1p.tile([128, TH], f16, tag="t1")
                nc.vector.tensor_mul(t1[:], xt[h][cc][:], rstd_b[:])
                t2 = t1p.tile([128, TH], f16, tag="t1b")
                nc.vector.tensor_add(t2[:], t1[:], nmr_b[:])
                x2 = x2p.tile([128, TH], f16, tag="x2")
                nc.scalar.activation(x2[:], t2[:], AF.Identity,
                                     bias=bf_c[:, cc:cc + 1])
                out.append(x2)
            return out

        hf = [None, None]
        mark("L0:ln1A")
        x2_next_A = emit_ln(0)      # layer-0 LN1(A) straight from x0
        for l in range(L):
            mark(f"L{l}:w")
            LW["wq"] = []
            for cc in range(NCH):
                t = wqp.tile([128, 3 * QO], f16, tag="wq")
                nc.sync.dma_start(t[:], wqkv_d[l, cc * 128:(cc + 1) * 128, :])
                LW["wq"].append(t)
            LW["w1"], LW["w2"] = [], []
            for cc in range(NCH):
                t = w1p.tile([128, FL], f16, tag="w1")
                nc.sync.dma_start(t[:], w1_d[l, cc * 128:(cc + 1) * 128, :])
                LW["w1"].append(t)
                t = w2p.tile([128, C], f16, tag="w2")
                nc.sync.dma_start(t[:], w2_d[l, cc * 128:(cc + 1) * 128, :])
                LW["w2"].append(t)

            mark(f"L{l}:qkvA")
            emit_qkv(l, 0, x2_next_A)
            mark(f"L{l}:scoA")
            ybfA = [None, None]
            for hh in range(HL):
                emit_scores_head(0, hh)
            mark(f"L{l}:ln1B")
            if l > 0:
                emit_resm(1)
            x2B = emit_ln(1)
            mark(f"L{l}:qkvB")
            emit_qkv(l, 1, x2B)
            mark(f"L{l}:avA")
            for hh in range(HL):
                emit_av_head(l, 0, hh, ybfA)
            emit_ag(0, ybfA)
            mark(f"L{l}:attB01")
            ybfB = [None, None]
            emit_scores_head(1, 0)
            emit_scores_head(1, 1)
            emit_av_head(l, 1, 0, ybfB)
            emit_av_head(l, 1, 1, ybfB)
            emit_scores_head(1, 2)
            emit_scores_head(1, 3)
            mark(f"L{l}:ln2A")
            emit_resy(0)
            x2mA = emit_ln(0)
            mark(f"L{l}:attB23")
            emit_av_head(l, 1, 2, ybfB)
            emit_av_head(l, 1, 3, ybfB)
            emit_ag(1, ybfB)
            mark(f"L{l}:mlp1A")
            a_A = emit_mlp1(l, 0, x2mA)
            mark(f"L{l}:mlp2A")
            emit_mlp2_ar(l, 0, a_A)
            mark(f"L{l}:ln2B")
            emit_resy(1)
            x2mB = emit_ln(1)
            mark(f"L{l}:mlp1B")
            a_B = emit_mlp1(l, 1, x2mB)
            mark(f"L{l}:preA")
            emit_resm(0)            # AR(A) of this layer just landed
            if l < L - 1:
                x2_next_A = emit_ln(0)
            else:
                hf[0] = emit_fin(0)
                for vb in range(2):     # prefetch head vb0/vb1 weights
                    for cc in range(NCH):
                        wt = hwp.tile([128, 512], f16, tag="hw")
                        nc.sync.dma_start(wt[:, 0:512],
                                          hw_d[cc * 128:(cc + 1) * 128,
                                               vb * 512:vb * 512 + 512])
                        S[("hw", vb, cc)] = wt
            mark(f"L{l}:mlp2B")
            emit_mlp2_ar(l, 1, a_B)

        mark("finB")
        emit_resm(1)
        hf[1] = emit_fin(1)

        NVB = (VL + 511) // 512

        def load_hw(vb):
            vn = min(512, VL - vb * 512)
            ts = []
            for cc in range(NCH):
                wt = hwp.tile([128, 512], f16, tag="hw")
                nc.sync.dma_start(wt[:, 0:vn],
                                  hw_d[cc * 128:(cc + 1) * 128,
                                       vb * 512:vb * 512 + vn])
                ts.append(wt)
            return ts

        pend = [[S[("hw", vb, cc)] for cc in range(NCH)] for vb in range(2)]
        PSH = [psA, psB, psC]
        for vb in range(NVB):
            vn = min(512, VL - vb * 512)
            rhs_t = pend.pop(0)
            if vb + 2 < NVB:
                pend.append(load_hw(vb + 2))
            for tcc in range(8):
                h, tl = tcc // 4, tcc % 4
                pool = PSH[tcc % 3]
                ph = pool.tile([128, 512], f32,
                               tag={0: "pmm", 1: "pa", 2: "py"}[tcc % 3])
                for cc in range(NCH):
                    nc.tensor.matmul(ph[:, 0:vn],
                                     hf[h][cc][:, tl * 128:(tl + 1) * 128],
                                     rhs_t[cc][:, 0:vn],
                                     start=(cc == 0), stop=(cc == NCH - 1))
                so = outp.tile([128, 512], f16, tag="so")
                if tcc % 2:
                    nc.vector.tensor_copy(so[:, 0:vn], ph[:, 0:vn])
                else:
                    nc.scalar.activation(so[:, 0:vn], ph[:, 0:vn], AF.Copy)
                nc.sync.dma_start(out_d[tcc * 128:(tcc + 1) * 128,
                                        vb * 512:vb * 512 + vn],
                                  so[:, 0:vn])
            mark(f"head{vb}")


def _prep_inputs(idx, tok_emb, pos_emb, ln1_w, ln1_b, wq, bq, wk, bk, wv, bv,
                 ln2_w, ln2_b, w1, b1, w2, b2, lnf_w, lnf_b, head_w):
    fh = np.float16

    mask = np.zeros((128, 128), np.float32)
    p, t = np.meshgrid(np.arange(128), np.arange(128), indexing="ij")
    mask[p <= t] = 1.0
    mask = mask.astype(ml_dtypes.bfloat16)

    x0s = []
    for g in range(B):
        x0 = tok_emb[np.asarray(idx[g], np.int64)] + pos_emb[0]
        x0s.append(np.ascontiguousarray(x0.T, np.float32).astype(fh))

    ln1w = np.asarray(ln1_w, np.float32)
    ln1b = np.asarray(ln1_b, np.float32)
    ln2w = np.asarray(ln2_w, np.float32)
    ln2b = np.asarray(ln2_b, np.float32)

    in_maps = []
    shard_cache = {}
    for c in range(8):
        g, j = c // 4, c % 4
        if j in shard_cache:
            m = dict(shard_cache[j])
            m["x0t"] = x0s[g]
            in_maps.append(m)
            continue
        sl = slice(j * QO, (j + 1) * QO)
        wq_j = np.asarray(wq[:, :, sl], np.float32)
        wk_j = np.asarray(wk[:, :, sl], np.float32)
        wv_j = np.asarray(wv[:, :, sl], np.float32)
        w1_j = np.asarray(w1[:, :, j * FL:(j + 1) * FL], np.float32)
        w2_j = np.asarray(w2[:, j * FL:(j + 1) * FL, :], np.float32)
        hw_j = np.asarray(head_w[:, j * VL:(j + 1) * VL], np.float32)

        # fold LN scale into the consuming weights
        wqkv = np.concatenate([wq_j, wk_j, wv_j], axis=2) * ln1w[:, :, None]
        w1f = w1_j * ln2w[:, :, None]
        hwf = hw_j * np.asarray(lnf_w, np.float32)[:, None]

        # bias columns: layer bias + W^T ln_b (the folded-LN bias term)
        # bqk: [128, L*4], col l*4 + oc covers q0,q1,k0,k1
        bqk = np.zeros((L, 4, 128), np.float32)
        byc = np.zeros((L, 2, 128), np.float32)
        for l in range(L):
            q_eff = bq[l, sl] + wq_j[l].T @ ln1b[l]
            k_eff = bk[l, sl] + wk_j[l].T @ ln1b[l]
            v_eff = bv[l, sl] + wv_j[l].T @ ln1b[l]
            bqk[l, 0] = q_eff[0:128]
            bqk[l, 1] = q_eff[128:256]
            bqk[l, 2] = k_eff[0:128]
            bqk[l, 3] = k_eff[128:256]
            byc[l, 0] = v_eff[0:128]
            byc[l, 1] = v_eff[128:256]
        bqk = np.ascontiguousarray(bqk.reshape(L * 4, 128).T)
        byc = np.ascontiguousarray(byc.reshape(L * 2, 128).T)

        b1c = np.zeros((L, NCH, 128), np.float32)
        for l in range(L):
            f_eff = b1[l, j * FL:(j + 1) * FL] + w1_j[l].T @ ln2b[l]
            b1c[l] = f_eff.reshape(NCH, 128)
        b1c = np.ascontiguousarray(b1c.reshape(L * 8, 128).T)

        b2c = np.ascontiguousarray(
            (np.asarray(b2, np.float32) / TP).reshape(L, NCH, 128)
            .reshape(L * 8, 128).T)

        lw = np.asarray(lnf_w, np.float32)
        lb = np.asarray(lnf_b, np.float32)
        bfc = np.where(np.abs(lw) > 1e-12, lb / np.where(lw == 0, 1, lw), 0.0)
        bfc = np.ascontiguousarray(bfc.reshape(NCH, 128).T.astype(np.float32))

        m = {
            "x0t": x0s[g],
            "wqkv": np.ascontiguousarray(wqkv).astype(fh),
            "w1": np.ascontiguousarray(w1f).astype(fh),
            "w2": np.ascontiguousarray(w2_j).astype(fh),
            "hw": np.ascontiguousarray(hwf).astype(fh),
            "bqk": bqk, "byc": byc, "b1c": b1c, "b2c": b2c, "bfc": bfc,
            "mask": mask,
        }
        shard_cache[j] = m
        in_maps.append(m)
    return in_maps


def kernel(**inputs):
    if "nc" not in _STATE:
        _STATE["nc"] = _build()
    nc = _STATE["nc"]
    in_maps = _prep_inputs(**{k: np.asarray(v) for k, v in inputs.items()})
    res = bass_utils.run_bass_kernel_spmd(nc, in_maps, core_ids=list(range(8)))
    outs = res.results
    full = np.empty((B, T, V), np.float32)
    for c in range(8):
        g, j = c // 4, c % 4
        full[g, :, j * VL:(j + 1) * VL] = np.asarray(outs[c]["out"],
                                                     np.float32)
    return full


# revision 55
# speedup vs baseline: 1.6041x; 1.0561x over previous
"""GPT forward (8 layers, C=1024, T=1024, B=2, H=16, V=32000) on 8 trn2 cores.

Sharding: TP4 x DP2. Cores 0-3 handle batch 0, cores 4-7 batch 1.
Within a quad, core j owns heads 4j..4j+3, MLP hidden slice j*1024..,
and vocab slice j*8000.. of the LM head.

v2 design notes:
- Residual stream lives in SBUF as fp16, transposed ([C, T] with channels
  on partitions), split into two T/2 token halves that are software-
  pipelined through the whole network so collectives overlap compute.
- LayerNorm is folded into the weights on the host (W <- W * ln_w), and
  all biases (ln_b contributions + layer biases) ride the existing
  PSUM->SBUF casts as per-partition bias columns, so normalization on
  device is only: stats matmuls (ones-vector trick, fp16 at 1 cyc/row),
  a tiny per-token scalar chain, and x2 = x*rstd + (-mu*rstd) per chunk.
- Attention is max-free softmax (exp then divide by the ones-column
  denominator folded into the AV matmul), with causality exploited at
  128-column granularity (ragged score matmuls, triangular AV chains).
- Collectives (y AllGather, MLP AllReduce) are fp16, per half, and are
  covered by the other half's compute in the pipeline.
"""

import numpy as np
import ml_dtypes

import concourse.bacc as bacc
import concourse.bass as bass
import concourse.tile as tile
import concourse.mybir as mybir
from concourse import bass_utils

f32 = mybir.dt.float32
f16 = mybir.dt.float16
bf16 = mybir.dt.bfloat16
AF = mybir.ActivationFunctionType
OP = mybir.AluOpType

B, T, C, L, H, F, V = 2, 1024, 1024, 8, 16, 4096, 32000
HD = C // H            # 64
TP = 4                 # tensor-parallel within a quad
HL = H // TP           # 4 local heads
QO = C // TP           # 256 local q/k/v width
FL = F // TP           # 1024 local mlp hidden
VL = V // TP           # 8000 local vocab
NCH = C // 128         # 8 channel chunks
TH = T // 2            # 512 tokens per half
GROUPS = [[0, 1, 2, 3], [4, 5, 6, 7]]
LN_EPS = 1e-5
SCALE = 1.0 / np.sqrt(HD)

_STATE = {}
_PHASE_LOG = []


def _steer_act_tables(arch):
    """The act-table-load pass greedily picks the first table set containing
    a function; `natural_log` (ln-only) shadows `natural_log_exp_and_others`,
    forcing a reload on every Ln->Exp pair in the LN chain. Empty the ln-only
    set (names and indices stay valid) so ln and exp share one table."""
    import concourse.hw_specs as hw_specs
    try:
        tables = hw_specs.get_activation_tables(arch)
        for name in ("natural_log",):
            if name in tables:
                tables[name].clear()
    except Exception:
        pass


def _build(collectives=True):
    nc = bacc.Bacc("TRN2", target_bir_lowering=False, debug=False,
                   enable_asserts=False, num_devices=8)
    _steer_act_tables(nc.m.arch)

    x0T_d = nc.dram_tensor("x0t", [C, T], f16, kind="ExternalInput").ap()
    wqkv_d = nc.dram_tensor("wqkv", [L, C, 3 * QO], f16, kind="ExternalInput").ap()
    w1_d = nc.dram_tensor("w1", [L, C, FL], f16, kind="ExternalInput").ap()
    w2_d = nc.dram_tensor("w2", [L, FL, C], f16, kind="ExternalInput").ap()
    hw_d = nc.dram_tensor("hw", [C, VL], f16, kind="ExternalInput").ap()
    # per-partition bias columns (all layer biases + folded-LN bias terms)
    bqk_d = nc.dram_tensor("bqk", [128, L * 4], f32, kind="ExternalInput").ap()
    by_d = nc.dram_tensor("byc", [128, L * 2], f32, kind="ExternalInput").ap()
    b1_d = nc.dram_tensor("b1c", [128, L * 8], f32, kind="ExternalInput").ap()
    b2_d = nc.dram_tensor("b2c", [128, L * 8], f32, kind="ExternalInput").ap()
    bf_d = nc.dram_tensor("bfc", [128, 8], f32, kind="ExternalInput").ap()
    mask_d = nc.dram_tensor("mask", [128, 128], bf16, kind="ExternalInput").ap()
    out_d = nc.dram_tensor("out", [T, VL], f16, kind="ExternalOutput").ap()

    with tile.TileContext(nc) as tc:
        _prog(nc, tc, x0T_d, wqkv_d, w1_d, w2_d, hw_d, bqk_d, by_d, b1_d,
              b2_d, bf_d, mask_d, out_d, collectives)
    nc.compile()
    return nc


def _prog(nc, tc, x0T_d, wqkv_d, w1_d, w2_d, hw_d, bqk_d, by_d, b1_d, b2_d,
          bf_d, mask_d, out_d, collectives=True):
    def mark(label):
        _PHASE_LOG.append((int(nc.next_id()), label))
    import contextlib
    ctx = contextlib.ExitStack()
    with ctx:
        const = ctx.enter_context(tc.tile_pool(name="const", bufs=1))
        xp = ctx.enter_context(tc.tile_pool(name="xres", bufs=1))
        x2p = ctx.enter_context(tc.tile_pool(name="x2", bufs=17))
        t1p = ctx.enter_context(tc.tile_pool(name="t1", bufs=2))
        sqp = ctx.enter_context(tc.tile_pool(name="sq", bufs=2))
        qkp = ctx.enter_context(tc.tile_pool(name="qk", bufs=4))
        vp = ctx.enter_context(tc.tile_pool(name="vsb", bufs=9))
        abp = ctx.enter_context(tc.tile_pool(name="ab", bufs=18))
        ap_ = ctx.enter_context(tc.tile_pool(name="act", bufs=10))
        yrp = ctx.enter_context(tc.tile_pool(name="yraw", bufs=2))
        ybp = ctx.enter_context(tc.tile_pool(name="ybf", bufs=4))
        bcp = ctx.enter_context(tc.tile_pool(name="bcast", bufs=4))
        dbp = ctx.enter_context(tc.tile_pool(name="denb", bufs=2))
        mop = ctx.enter_context(tc.tile_pool(name="mo", bufs=3))
        rbp = ctx.enter_context(tc.tile_pool(name="rb", bufs=2))
        smp = ctx.enter_context(tc.tile_pool(name="small", bufs=8))
        wqp = ctx.enter_context(tc.tile_pool(name="wqkv", bufs=14))
        w1p = ctx.enter_context(tc.tile_pool(name="w1", bufs=8))
        w2p = ctx.enter_context(tc.tile_pool(name="w2", bufs=8))
        hwp = ctx.enter_context(tc.tile_pool(name="hwsb", bufs=11))
        outp = ctx.enter_context(tc.tile_pool(name="outs", bufs=3))
        psA = ctx.enter_context(tc.tile_pool(name="psA", bufs=2, space="PSUM"))
        psB = ctx.enter_context(tc.tile_pool(name="psB", bufs=2, space="PSUM"))
        psC = ctx.enter_context(tc.tile_pool(name="psC", bufs=2, space="PSUM"))
        psS = ctx.enter_context(tc.tile_pool(name="psS", bufs=2, space="PSUM"))
        dr = ctx.enter_context(tc.tile_pool(name="dram", bufs=2, space="DRAM"))

        ones16 = const.tile([128, 1], f16)
        nc.vector.memset(ones16[:], 1.0)
        eps_t = const.tile([1, 1], f32, tag="eps")
        nc.vector.memset(eps_t[:], C * LN_EPS)
        lnc_t = const.tile([1, 1], f32, tag="lnc")
        nc.vector.memset(lnc_t[:], 0.5 * float(np.log(C)))
        mask = const.tile([128, 128], bf16)
        nc.sync.dma_start(mask[:], mask_d[:])
        bqk_c = const.tile([128, L * 4], f32, tag="bqk")
        nc.sync.dma_start(bqk_c[:], bqk_d[:])
        by_c = const.tile([128, L * 2], f32, tag="byc")
        nc.sync.dma_start(by_c[:], by_d[:])
        b1_c = const.tile([128, L * 8], f32, tag="b1c")
        nc.sync.dma_start(b1_c[:], b1_d[:])
        b2_c = const.tile([128, L * 8], f32, tag="b2c")
        nc.sync.dma_start(b2_c[:], b2_d[:])
        bf_c = const.tile([128, 8], f32, tag="bfc")
        nc.sync.dma_start(bf_c[:], bf_d[:])

        # residual stream: per (half, chunk) fp16 [128, 512], persistent
        xt = [[None] * NCH for _ in range(2)]
        for h in range(2):
            for cc in range(NCH):
                t = xp.tile([128, TH], f16, tag=f"x{h}_{cc}")
                nc.sync.dma_start(t[:], x0T_d[cc * 128:(cc + 1) * 128,
                                               h * TH:(h + 1) * TH])
                xt[h][cc] = t

        def ln_stats(h):
            """stats + per-token chain -> (rstd_b, nmr_b) fp16 [128, TH]."""
            ssum = psS.tile([1, TH], f32, tag="st")
            sqsum = psS.tile([1, TH], f32, tag="st")
            sqs = []
            for cc in range(NCH):
                sq = sqp.tile([128, TH], f16, tag="sq", bufs=6)
                nc.vector.tensor_mul(sq[:], xt[h][cc][:], xt[h][cc][:])
                sqs.append(sq)
                nc.tensor.matmul(ssum[:], ones16[:], xt[h][cc][:],
                                 start=(cc == 0), stop=(cc == NCH - 1))
            for cc in range(NCH):
                nc.tensor.matmul(sqsum[:], ones16[:], sqs[cc][:],
                                 start=(cc == 0), stop=(cc == NCH - 1))
            # rstd = (var+eps)^-1/2 via q = sqsum - ssum^2/C + C*eps:
            # rstd = exp(-0.5*ln(q) + 0.5*ln(C)); nmr = -(ssum/C)*rstd
            t = smp.tile([1, TH], f32, tag="sm")
            nc.scalar.activation(t[:], ssum[:], AF.Square)
            q = smp.tile([1, TH], f32, tag="sm")
            nc.vector.scalar_tensor_tensor(q[:], t[:], -1.0 / C, sqsum[:],
                                           op0=OP.mult, op1=OP.add)
            lnq = smp.tile([1, TH], f32, tag="sm")
            nc.scalar.activation(lnq[:], q[:], AF.Ln, bias=eps_t[:])
            r16 = smp.tile([1, TH], f16, tag="sm16")
            nc.scalar.activation(r16[:], lnq[:], AF.Exp, scale=-0.5,
                                 bias=lnc_t[:])
            n16 = smp.tile([1, TH], f16, tag="sm16")
            nc.vector.scalar_tensor_tensor(n16[:], ssum[:], -1.0 / C, r16[:],
                                           op0=OP.mult, op1=OP.mult)
            rstd_b = bcp.tile([128, TH], f16, tag="bc")
            nc.gpsimd.partition_broadcast(rstd_b[:], r16[:])
            nmr_b = bcp.tile([128, TH], f16, tag="bc")
            nc.gpsimd.partition_broadcast(nmr_b[:], n16[:])
            return rstd_b, nmr_b

        def ln_norm(h, rstd_b, nmr_b, cc):
            """x2 = x*rstd + nmr for one chunk -> fp16 tile."""
            t1 = t1p.tile([128, TH], f16, tag="t1")
            nc.vector.tensor_mul(t1[:], xt[h][cc][:], rstd_b[:])
            x2 = x2p.tile([128, TH], f16, tag="x2")
            nc.vector.tensor_add(x2[:], t1[:], nmr_b[:])
            return x2

        r_out_prev = [None, None]   # AR output dram tiles per half
        LW = {}                     # current layer's weight tiles
        S = {}                      # rolling attention state

        def emit_resm(h):
            for g in range(2):
                rt = rbp.tile([128, 4, TH], f16, tag="rb")
                nc.sync.dma_start(
                    rt[:, :, :],
                    r_out_prev[h][g * 4:(g + 1) * 4, :, :].transpose([1, 0, 2]))
                for k in range(4):
                    cc = g * 4 + k
                    nc.vector.tensor_add(xt[h][cc][:], xt[h][cc][:],
                                         rt[:, k, :])

        def emit_ln(h):
            rstd_b, nmr_b = ln_stats(h)
            return [ln_norm(h, rstd_b, nmr_b, cc) for cc in range(NCH)]

        def emit_qkv(l, h, x2):
            wq_t = LW["wq"]
            qk = []
            for oc in range(4):
                p = psA.tile([128, TH], f32, tag="pmm")
                for cc in range(NCH):
                    nc.tensor.matmul(p[:], wq_t[cc][:, oc * 128:(oc + 1) * 128],
                                     x2[cc][:],
                                     start=(cc == 0), stop=(cc == NCH - 1))
                dst = qkp.tile([128, TH], f16, tag=("qh" if oc < 2 else "kh"))
                nc.vector.tensor_scalar_add(
                    dst[:], p[:], bqk_c[:, l * 4 + oc:l * 4 + oc + 1])
                qk.append(dst)
            vts = []
            for tv in range(4):
                pv = psA.tile([128, 4, HD], f32, tag="pmm")
                for cc in range(NCH):
                    nc.tensor.matmul(pv[:, :, :],
                                     x2[cc][:, tv * 128:(tv + 1) * 128],
                                     wq_t[cc][:, 2 * QO:3 * QO],
                                     start=(cc == 0), stop=(cc == NCH - 1))
                vt = vp.tile([128, 4, HD + 1], bf16, tag="v")
                nc.vector.memset(vt[:, :, HD:HD + 1], 1.0)
                nc.vector.tensor_copy(vt[:, :, 0:HD], pv[:, :, :])
                vts.append(vt)
            S[("q", h)] = qk[:2]
            S[("k", h)] = qk[2:]
            S[("v", h)] = vts

        def emit_scores_head(h, hh):
            qi, ro = hh // 2, (hh % 2) * 64
            for si in range(4 * h + 4):
                c0 = max(h * TH, si * 128)
                w = h * TH + TH - c0
                pool, ptag = ((psB, "pa") if si % 2 else (psA, "pmm"))
                pa = pool.tile([128, TH], f32, tag=ptag)
                nc.tensor.matmul(
                    pa[:, 0:w],
                    S[("k", si // 4)][qi][ro:ro + 64,
                                          (si % 4) * 128:(si % 4) * 128 + 128],
                    S[("q", h)][qi][ro:ro + 64, c0 - h * TH:c0 - h * TH + w],
                    start=True, stop=True)
                abt = abp.tile([128, TH], bf16, tag="ab")
                nc.scalar.activation(abt[:, 0:w], pa[:, 0:w], AF.Exp,
                                     scale=float(SCALE))
                if c0 == si * 128:
                    nc.vector.tensor_mul(abt[:, 0:128], abt[:, 0:128], mask[:])
                S[("ab", h, hh, si)] = (abt, c0)

        def emit_av_head(l, h, hh, ybf):
            py = psC.tile([HD + 1, TH], f32, tag="py")
            for tcl in range(4):
                tcg = 4 * h + tcl
                for si in range(tcg + 1):
                    abt, c0 = S[("ab", h, hh, si)]
                    nc.tensor.matmul(
                        py[:, tcl * 128:(tcl + 1) * 128],
                        S[("v", si // 4)][si % 4][:, hh:hh + 1, :],
                        abt[:, tcg * 128 - c0:tcg * 128 - c0 + 128],
                        start=(si == 0), stop=(si == tcg))
            den = smp.tile([1, TH], f32, tag="den", bufs=3)
            nc.vector.reciprocal(den[:], py[HD:HD + 1, :])
            den_b = dbp.tile([64, TH], f32, tag="db")
            nc.gpsimd.partition_broadcast(den_b[:], den[:])
            yraw = yrp.tile([64, TH], f16, tag="yr")
            nc.vector.tensor_mul(yraw[:], py[0:HD, :], den_b[:])
            i = hh // 2
            if ybf[i] is None:
                ybf[i] = ybp.tile([128, TH], f16, tag="yb",
                                  name=f"yb{l}_{h}_{i}")
            ro2 = (hh % 2) * 64
            nc.scalar.activation(ybf[i][ro2:ro2 + 64, :], yraw[:], AF.Identity,
                                 bias=by_c[ro2:ro2 + 64, l * 2 + i:l * 2 + i + 1])

        def emit_ag(h, ybf):
            g_in = dr.tile([QO, TH], f16, tag=f"gi{h}")
            for i in range(2):
                nc.sync.dma_start(g_in[i * 128:(i + 1) * 128, :], ybf[i][:])
            g_out = dr.tile([NCH, 128, TH], f16, tag=f"go{h}")
            if collectives is True:
                nc.gpsimd.collective_compute(
                    "AllGather", OP.bypass, replica_groups=GROUPS,
                    ins=[g_in.opt()], outs=[g_out.opt()])
            else:
                for q in range(TP):
                    nc.sync.dma_start(g_out[q * 2:(q + 1) * 2, :, :],
                                      g_in[:])
            S[("go", h)] = g_out

        def emit_resy(h):
            for g in range(2):
                yt = rbp.tile([128, 4, TH], f16, tag="rb")
                nc.sync.dma_start(
                    yt[:, :, :],
                    S[("go", h)][g * 4:(g + 1) * 4, :, :].transpose([1, 0, 2]))
                for k in range(4):
                    cc = g * 4 + k
                    nc.vector.tensor_add(xt[h][cc][:], xt[h][cc][:],
                                         yt[:, k, :])

        def emit_mlp1(l, h, x2):
            a_t = []
            for fc in range(NCH):
                pm = psA.tile([128, TH], f32, tag="pmm")
                for cc in range(NCH):
                    nc.tensor.matmul(pm[:],
                                     LW["w1"][cc][:, fc * 128:(fc + 1) * 128],
                                     x2[cc][:],
                                     start=(cc == 0), stop=(cc == NCH - 1))
                ga = ap_.tile([128, TH], f16, tag="a")
                nc.scalar.activation(ga[:], pm[:], AF.Gelu,
                                     bias=b1_c[:, l * 8 + fc:l * 8 + fc + 1])
                a_t.append(ga)
            return a_t

        def emit_mlp2_ar(l, h, a_t):
            r_in = dr.tile([NCH, 128, TH], f16, tag=f"ri{h}")
            for cc in range(NCH):
                pm2 = psA.tile([128, TH], f32, tag="pmm")
                for fc in range(NCH):
                    nc.tensor.matmul(pm2[:],
                                     LW["w2"][fc][:, cc * 128:(cc + 1) * 128],
                                     a_t[fc][:],
                                     start=(fc == 0), stop=(fc == NCH - 1))
                mo = mop.tile([128, TH], f16, tag="mo")
                nc.scalar.activation(mo[:], pm2[:], AF.Identity,
                                     bias=b2_c[:, l * 8 + cc:l * 8 + cc + 1])
                nc.sync.dma_start(r_in[cc, :, :], mo[:])
            r_out = dr.tile([NCH, 128, TH], f16, tag=f"ro{h}")
            if collectives is True:
                nc.gpsimd.collective_compute(
                    "AllReduce", OP.add, replica_groups=GROUPS,
                    ins=[r_in.opt()], outs=[r_out.opt()])
            else:
                for g in range(2):
                    nc.sync.dma_start(r_out[g * 4:(g + 1) * 4, :, :],
                                      r_in[g * 4:(g + 1) * 4, :, :])
            r_out_prev[h] = r_out

        def emit_fin(h):
            rstd_b, nmr_b = ln_stats(h)
            out = []
            for cc in range(NCH):
                t1 = t1p.tile([128, TH], f16, tag="t1")
                nc.vector.tensor_mul(t1[:], xt[h][cc][:], rstd_b[:])
                t2 = t1p.tile([128, TH], f16, tag="t1b")
                nc.vector.tensor_add(t2[:], t1[:], nmr_b[:])
                x2 = x2p.tile([128, TH], f16, tag="x2")
                nc.scalar.activation(x2[:], t2[:], AF.Identity,
                                     bias=bf_c[:, cc:cc + 1])
                out.append(x2)
            return out

        hf = [None, None]
        mark("L0:ln1A")
        x2_next_A = emit_ln(0)      # layer-0 LN1(A) straight from x0
        for l in range(L):
            mark(f"L{l}:w")
            LW["wq"] = []
            for cc in range(NCH):
                t = wqp.tile([128, 3 * QO], f16, tag="wq")
                nc.sync.dma_start(t[:], wqkv_d[l, cc * 128:(cc + 1) * 128, :])
                LW["wq"].append(t)
            LW["w1"], LW["w2"] = [], []
            for cc in range(NCH):
                t = w1p.tile([128, FL], f16, tag="w1")
                nc.sync.dma_start(t[:], w1_d[l, cc * 128:(cc + 1) * 128, :])
                LW["w1"].append(t)
                t = w2p.tile([128, C], f16, tag="w2")
                nc.sync.dma_start(t[:], w2_d[l, cc * 128:(cc + 1) * 128, :])
                LW["w2"].append(t)

            mark(f"L{l}:qkvA")
            emit_qkv(l, 0, x2_next_A)
            mark(f"L{l}:scoA")
            ybfA = [None, None]
            for hh in range(HL):
                emit_scores_head(0, hh)
            mark(f"L{l}:ln1B")
            if l > 0:
                emit_resm(1)
            x2B = emit_ln(1)
            mark(f"L{l}:avA")
            emit_av_head(l, 0, 0, ybfA)
            emit_av_head(l, 0, 1, ybfA)
            mark(f"L{l}:qkvB")
            emit_qkv(l, 1, x2B)
            mark(f"L{l}:avA2")
            emit_av_head(l, 0, 2, ybfA)
            emit_av_head(l, 0, 3, ybfA)
            emit_ag(0, ybfA)
            mark(f"L{l}:attB01")
            ybfB = [None, None]
            emit_scores_head(1, 0)
            emit_scores_head(1, 1)
            emit_av_head(l, 1, 0, ybfB)
            emit_av_head(l, 1, 1, ybfB)
            emit_scores_head(1, 2)
            emit_scores_head(1, 3)
            mark(f"L{l}:ln2A")
            emit_resy(0)
            x2mA = emit_ln(0)
            mark(f"L{l}:attB23")
            emit_av_head(l, 1, 2, ybfB)
            emit_av_head(l, 1, 3, ybfB)
            emit_ag(1, ybfB)
            mark(f"L{l}:mlp1A")
            a_A = emit_mlp1(l, 0, x2mA)
            mark(f"L{l}:mlp2A")
            emit_mlp2_ar(l, 0, a_A)
            mark(f"L{l}:ln2B")
            emit_resy(1)
            x2mB = emit_ln(1)
            mark(f"L{l}:mlp1B")
            a_B = emit_mlp1(l, 1, x2mB)
            mark(f"L{l}:preA")
            emit_resm(0)            # AR(A) of this layer just landed
            if l < L - 1:
                x2_next_A = emit_ln(0)
            else:
                hf[0] = emit_fin(0)
                for vb in range(2):     # prefetch head vb0/vb1 weights
                    for cc in range(NCH):
                        wt = hwp.tile([128, 512], f16, tag="hw")
                        nc.sync.dma_start(wt[:, 0:512],
                                          hw_d[cc * 128:(cc + 1) * 128,
                                               vb * 512:vb * 512 + 512])
                        S[("hw", vb, cc)] = wt
            mark(f"L{l}:mlp2B")
            emit_mlp2_ar(l, 1, a_B)

        mark("finB")
        emit_resm(1)
        hf[1] = emit_fin(1)

        NVB = (VL + 511) // 512

        def load_hw(vb):
            vn = min(512, VL - vb * 512)
            ts = []
            for cc in range(NCH):
                wt = hwp.tile([128, 512], f16, tag="hw")
                nc.sync.dma_start(wt[:, 0:vn],
                                  hw_d[cc * 128:(cc + 1) * 128,
                                       vb * 512:vb * 512 + vn])
                ts.append(wt)
            return ts

        pend = [[S[("hw", vb, cc)] for cc in range(NCH)] for vb in range(2)]
        PSH = [psA, psB, psC]
        for vb in range(NVB):
            vn = min(512, VL - vb * 512)
            rhs_t = pend.pop(0)
            if vb + 2 < NVB:
                pend.append(load_hw(vb + 2))
            for tcc in range(8):
                h, tl = tcc // 4, tcc % 4
                pool = PSH[tcc % 3]
                ph = pool.tile([128, 512], f32,
                               tag={0: "pmm", 1: "pa", 2: "py"}[tcc % 3])
                for cc in range(NCH):
                    nc.tensor.matmul(ph[:, 0:vn],
                                     hf[h][cc][:, tl * 128:(tl + 1) * 128],
                                     rhs_t[cc][:, 0:vn],
                                     start=(cc == 0), stop=(cc == NCH - 1))
                so = outp.tile([128, 512], f16, tag="so")
                if tcc % 2:
                    nc.vector.tensor_copy(so[:, 0:vn], ph[:, 0:vn])
                else:
                    nc.scalar.activation(so[:, 0:vn], ph[:, 0:vn], AF.Copy)
                nc.sync.dma_start(out_d[tcc * 128:(tcc + 1) * 128,
                                        vb * 512:vb * 512 + vn],
                                  so[:, 0:vn])
            mark(f"head{vb}")


def _prep_inputs(idx, tok_emb, pos_emb, ln1_w, ln1_b, wq, bq, wk, bk, wv, bv,
                 ln2_w, ln2_b, w1, b1, w2, b2, lnf_w, lnf_b, head_w):
    fh = np.float16

    mask = np.zeros((128, 128), np.float32)
    p, t = np.meshgrid(np.arange(128), np.arange(128), indexing="ij")
    mask[p <= t] = 1.0
    mask = mask.astype(ml_dtypes.bfloat16)

    x0s = []
    for g in range(B):
        x0 = tok_emb[np.asarray(idx[g], np.int64)] + pos_emb[0]
        x0s.append(np.ascontiguousarray(x0.T, np.float32).astype(fh))

    ln1w = np.asarray(ln1_w, np.float32)
    ln1b = np.asarray(ln1_b, np.float32)
    ln2w = np.asarray(ln2_w, np.float32)
    ln2b = np.asarray(ln2_b, np.float32)

    in_maps = []
    shard_cache = {}
    for c in range(8):
        g, j = c // 4, c % 4
        if j in shard_cache:
            m = dict(shard_cache[j])
            m["x0t"] = x0s[g]
            in_maps.append(m)
            continue
        sl = slice(j * QO, (j + 1) * QO)
        wq_j = np.asarray(wq[:, :, sl], np.float32)
        wk_j = np.asarray(wk[:, :, sl], np.float32)
        wv_j = np.asarray(wv[:, :, sl], np.float32)
        w1_j = np.asarray(w1[:, :, j * FL:(j + 1) * FL], np.float32)
        w2_j = np.asarray(w2[:, j * FL:(j + 1) * FL, :], np.float32)
        hw_j = np.asarray(head_w[:, j * VL:(j + 1) * VL], np.float32)

        # fold LN scale into the consuming weights
        wqkv = np.concatenate([wq_j, wk_j, wv_j], axis=2) * ln1w[:, :, None]
        w1f = w1_j * ln2w[:, :, None]
        hwf = hw_j * np.asarray(lnf_w, np.float32)[:, None]

        # bias columns: layer bias + W^T ln_b (the folded-LN bias term)
        # bqk: [128, L*4], col l*4 + oc covers q0,q1,k0,k1
        bqk = np.zeros((L, 4, 128), np.float32)
        byc = np.zeros((L, 2, 128), np.float32)
        for l in range(L):
            q_eff = bq[l, sl] + wq_j[l].T @ ln1b[l]
            k_eff = bk[l, sl] + wk_j[l].T @ ln1b[l]
            v_eff = bv[l, sl] + wv_j[l].T @ ln1b[l]
            bqk[l, 0] = q_eff[0:128]
            bqk[l, 1] = q_eff[128:256]
            bqk[l, 2] = k_eff[0:128]
            bqk[l, 3] = k_eff[128:256]
            byc[l, 0] = v_eff[0:128]
            byc[l, 1] = v_eff[128:256]
        bqk = np.ascontiguousarray(bqk.reshape(L * 4, 128).T)
        byc = np.ascontiguousarray(byc.reshape(L * 2, 128).T)

        b1c = np.zeros((L, NCH, 128), np.float32)
        for l in range(L):
            f_eff = b1[l, j * FL:(j + 1) * FL] + w1_j[l].T @ ln2b[l]
            b1c[l] = f_eff.reshape(NCH, 128)
        b1c = np.ascontiguousarray(b1c.reshape(L * 8, 128).T)

        b2c = np.ascontiguousarray(
            (np.asarray(b2, np.float32) / TP).reshape(L, NCH, 128)
            .reshape(L * 8, 128).T)

        lw = np.asarray(lnf_w, np.float32)
        lb = np.asarray(lnf_b, np.float32)
        bfc = np.where(np.abs(lw) > 1e-12, lb / np.where(lw == 0, 1, lw), 0.0)
        bfc = np.ascontiguousarray(bfc.reshape(NCH, 128).T.astype(np.float32))

        m = {
            "x0t": x0s[g],
            "wqkv": np.ascontiguousarray(wqkv).astype(fh),
            "w1": np.ascontiguousarray(w1f).astype(fh),
            "w2": np.ascontiguousarray(w2_j).astype(fh),
            "hw": np.ascontiguousarray(hwf).astype(fh),
            "bqk": bqk, "byc": byc, "b1c": b1c, "b2c": b2c, "bfc": bfc,
            "mask": mask,
        }
        shard_cache[j] = m
        in_maps.append(m)
    return in_maps


def kernel(**inputs):
    if "nc" not in _STATE:
        _STATE["nc"] = _build()
    nc = _STATE["nc"]
    in_maps = _prep_inputs(**{k: np.asarray(v) for k, v in inputs.items()})
    res = bass_utils.run_bass_kernel_spmd(nc, in_maps, core_ids=list(range(8)))
    outs = res.results
    full = np.empty((B, T, V), np.float32)
    for c in range(8):
        g, j = c // 4, c % 4
        full[g, :, j * VL:(j + 1) * VL] = np.asarray(outs[c]["out"],
                                                     np.float32)
    return full


# revision 65
# speedup vs baseline: 1.6467x; 1.0265x over previous
"""GPT forward (8 layers, C=1024, T=1024, B=2, H=16, V=32000) on 8 trn2 cores.

Sharding: TP4 x DP2. Cores 0-3 handle batch 0, cores 4-7 batch 1.
Within a quad, core j owns heads 4j..4j+3, MLP hidden slice j*1024..,
and vocab slice j*8000.. of the LM head.

v2 design notes:
- Residual stream lives in SBUF as fp16, transposed ([C, T] with channels
  on partitions), split into two T/2 token halves that are software-
  pipelined through the whole network so collectives overlap compute.
- LayerNorm is folded into the weights on the host (W <- W * ln_w), and
  all biases (ln_b contributions + layer biases) ride the existing
  PSUM->SBUF casts as per-partition bias columns, so normalization on
  device is only: stats matmuls (ones-vector trick, fp16 at 1 cyc/row),
  a tiny per-token scalar chain, and x2 = x*rstd + (-mu*rstd) per chunk.
- Attention is max-free softmax (exp then divide by the ones-column
  denominator folded into the AV matmul), with causality exploited at
  128-column granularity (ragged score matmuls, triangular AV chains).
- Collectives (y AllGather, MLP AllReduce) are fp16, per half, and are
  covered by the other half's compute in the pipeline.
"""

import numpy as np
import ml_dtypes

import concourse.bacc as bacc
import concourse.bass as bass
import concourse.tile as tile
import concourse.mybir as mybir
from concourse import bass_utils

f32 = mybir.dt.float32
f16 = mybir.dt.float16
bf16 = mybir.dt.bfloat16
AF = mybir.ActivationFunctionType
OP = mybir.AluOpType

B, T, C, L, H, F, V = 2, 1024, 1024, 8, 16, 4096, 32000
HD = C // H            # 64
TP = 4                 # tensor-parallel within a quad
HL = H // TP           # 4 local heads
QO = C // TP           # 256 local q/k/v width
FL = F // TP           # 1024 local mlp hidden
VL = V // TP           # 8000 local vocab
NCH = C // 128         # 8 channel chunks
TH = T // 2            # 512 tokens per half
GROUPS = [[0, 1, 2, 3], [4, 5, 6, 7]]
LN_EPS = 1e-5
SCALE = 1.0 / np.sqrt(HD)

_STATE = {}
_PHASE_LOG = []


def _steer_act_tables(arch):
    """The act-table-load pass greedily picks the first table set containing
    a function; `natural_log` (ln-only) shadows `natural_log_exp_and_others`,
    forcing a reload on every Ln->Exp pair in the LN chain. Empty the ln-only
    set (names and indices stay valid) so ln and exp share one table."""
    import concourse.hw_specs as hw_specs
    try:
        tables = hw_specs.get_activation_tables(arch)
        for name in ("natural_log",):
            if name in tables:
                tables[name].clear()
    except Exception:
        pass


def _build(collectives=True):
    nc = bacc.Bacc("TRN2", target_bir_lowering=False, debug=False,
                   enable_asserts=False, num_devices=8)
    _steer_act_tables(nc.m.arch)

    x0T_d = nc.dram_tensor("x0t", [C, T], f16, kind="ExternalInput").ap()
    wqkv_d = nc.dram_tensor("wqkv", [L, C, 3 * QO], f16, kind="ExternalInput").ap()
    w1_d = nc.dram_tensor("w1", [L, C, FL], f16, kind="ExternalInput").ap()
    w2_d = nc.dram_tensor("w2", [L, FL, C], f16, kind="ExternalInput").ap()
    hw_d = nc.dram_tensor("hw", [C, VL], f16, kind="ExternalInput").ap()
    # per-partition bias columns (all layer biases + folded-LN bias terms)
    bqk_d = nc.dram_tensor("bqk", [128, L * 4], f32, kind="ExternalInput").ap()
    by_d = nc.dram_tensor("byc", [128, L * 2], f32, kind="ExternalInput").ap()
    b1_d = nc.dram_tensor("b1c", [128, L * 8], f32, kind="ExternalInput").ap()
    b2_d = nc.dram_tensor("b2c", [128, L * 8], f32, kind="ExternalInput").ap()
    bf_d = nc.dram_tensor("bfc", [128, 8], f32, kind="ExternalInput").ap()
    mask_d = nc.dram_tensor("mask", [128, 128], bf16, kind="ExternalInput").ap()
    out_d = nc.dram_tensor("out", [T, VL], f16, kind="ExternalOutput").ap()

    with tile.TileContext(nc) as tc:
        _prog(nc, tc, x0T_d, wqkv_d, w1_d, w2_d, hw_d, bqk_d, by_d, b1_d,
              b2_d, bf_d, mask_d, out_d, collectives)
    nc.compile()
    return nc


def _prog(nc, tc, x0T_d, wqkv_d, w1_d, w2_d, hw_d, bqk_d, by_d, b1_d, b2_d,
          bf_d, mask_d, out_d, collectives=True):
    def mark(label):
        _PHASE_LOG.append((int(nc.next_id()), label))
    import contextlib
    ctx = contextlib.ExitStack()
    with ctx:
        const = ctx.enter_context(tc.tile_pool(name="const", bufs=1))
        xp = ctx.enter_context(tc.tile_pool(name="xres", bufs=1))
        x2p = ctx.enter_context(tc.tile_pool(name="x2", bufs=17))
        t1p = ctx.enter_context(tc.tile_pool(name="t1", bufs=2))
        sqp = ctx.enter_context(tc.tile_pool(name="sq", bufs=2))
        qkp = ctx.enter_context(tc.tile_pool(name="qk", bufs=4))
        vp = ctx.enter_context(tc.tile_pool(name="vsb", bufs=9))
        abp = ctx.enter_context(tc.tile_pool(name="ab", bufs=18))
        ap_ = ctx.enter_context(tc.tile_pool(name="act", bufs=10))
        yrp = ctx.enter_context(tc.tile_pool(name="yraw", bufs=2))
        ybp = ctx.enter_context(tc.tile_pool(name="ybf", bufs=4))
        bcp = ctx.enter_context(tc.tile_pool(name="bcast", bufs=4))
        dbp = ctx.enter_context(tc.tile_pool(name="denb", bufs=2))
        mop = ctx.enter_context(tc.tile_pool(name="mo", bufs=3))
        rbp = ctx.enter_context(tc.tile_pool(name="rb", bufs=2))
        smp = ctx.enter_context(tc.tile_pool(name="small", bufs=8))
        wqp = ctx.enter_context(tc.tile_pool(name="wqkv", bufs=14))
        w1p = ctx.enter_context(tc.tile_pool(name="w1", bufs=8))
        w2p = ctx.enter_context(tc.tile_pool(name="w2", bufs=8))
        hwp = ctx.enter_context(tc.tile_pool(name="hwsb", bufs=11))
        outp = ctx.enter_context(tc.tile_pool(name="outs", bufs=3))
        psA = ctx.enter_context(tc.tile_pool(name="psA", bufs=2, space="PSUM"))
        psB = ctx.enter_context(tc.tile_pool(name="psB", bufs=2, space="PSUM"))
        psC = ctx.enter_context(tc.tile_pool(name="psC", bufs=2, space="PSUM"))
        psS = ctx.enter_context(tc.tile_pool(name="psS", bufs=2, space="PSUM"))
        dr = ctx.enter_context(tc.tile_pool(name="dram", bufs=2, space="DRAM"))

        ones16 = const.tile([128, 1], f16)
        nc.vector.memset(ones16[:], 1.0)
        eps_t = const.tile([1, 1], f32, tag="eps")
        nc.vector.memset(eps_t[:], C * LN_EPS)
        lnc_t = const.tile([1, 1], f32, tag="lnc")
        nc.vector.memset(lnc_t[:], 0.5 * float(np.log(C)))
        mask = const.tile([128, 128], bf16)
        nc.sync.dma_start(mask[:], mask_d[:])
        bqk_c = const.tile([128, L * 4], f32, tag="bqk")
        nc.sync.dma_start(bqk_c[:], bqk_d[:])
        by_c = const.tile([128, L * 2], f32, tag="byc")
        nc.sync.dma_start(by_c[:], by_d[:])
        b1_c = const.tile([128, L * 8], f32, tag="b1c")
        nc.sync.dma_start(b1_c[:], b1_d[:])
        b2_c = const.tile([128, L * 8], f32, tag="b2c")
        nc.sync.dma_start(b2_c[:], b2_d[:])
        bf_c = const.tile([128, 8], f32, tag="bfc")
        nc.sync.dma_start(bf_c[:], bf_d[:])

        # residual stream: per (half, chunk) fp16 [128, 512], persistent
        xt = [[None] * NCH for _ in range(2)]
        for h in range(2):
            for cc in range(NCH):
                t = xp.tile([128, TH], f16, tag=f"x{h}_{cc}")
                nc.sync.dma_start(t[:], x0T_d[cc * 128:(cc + 1) * 128,
                                               h * TH:(h + 1) * TH])
                xt[h][cc] = t

        def ln_stats(h):
            """stats + per-token chain -> (rstd_b, nmr_b) fp16 [128, TH]."""
            ssum = psS.tile([1, TH], f32, tag="st")
            sqsum = psS.tile([1, TH], f32, tag="st")
            sqs = []
            for cc in range(NCH):
                sq = sqp.tile([128, TH], f16, tag="sq", bufs=6)
                nc.vector.tensor_mul(sq[:], xt[h][cc][:], xt[h][cc][:])
                sqs.append(sq)
                nc.tensor.matmul(ssum[:], ones16[:], xt[h][cc][:],
                                 start=(cc == 0), stop=(cc == NCH - 1))
            for cc in range(NCH):
                nc.tensor.matmul(sqsum[:], ones16[:], sqs[cc][:],
                                 start=(cc == 0), stop=(cc == NCH - 1))
            # rstd = (var+eps)^-1/2 via q = sqsum - ssum^2/C + C*eps:
            # rstd = exp(-0.5*ln(q) + 0.5*ln(C)); nmr = -(ssum/C)*rstd
            t = smp.tile([1, TH], f32, tag="sm")
            nc.scalar.activation(t[:], ssum[:], AF.Square)
            q = smp.tile([1, TH], f32, tag="sm")
            nc.vector.scalar_tensor_tensor(q[:], t[:], -1.0 / C, sqsum[:],
                                           op0=OP.mult, op1=OP.add)
            lnq = smp.tile([1, TH], f32, tag="sm")
            nc.scalar.activation(lnq[:], q[:], AF.Ln, bias=eps_t[:])
            r16 = smp.tile([1, TH], f16, tag="sm16")
            nc.scalar.activation(r16[:], lnq[:], AF.Exp, scale=-0.5,
                                 bias=lnc_t[:])
            n16 = smp.tile([1, TH], f16, tag="sm16")
            nc.vector.scalar_tensor_tensor(n16[:], ssum[:], -1.0 / C, r16[:],
                                           op0=OP.mult, op1=OP.mult)
            rstd_b = bcp.tile([128, TH], f16, tag="bc")
            nc.gpsimd.partition_broadcast(rstd_b[:], r16[:])
            nmr_b = bcp.tile([128, TH], f16, tag="bc")
            nc.gpsimd.partition_broadcast(nmr_b[:], n16[:])
            return rstd_b, nmr_b

        def ln_norm(h, rstd_b, nmr_b, cc):
            """x2 = x*rstd + nmr for one chunk -> fp16 tile."""
            t1 = t1p.tile([128, TH], f16, tag="t1")
            nc.vector.tensor_mul(t1[:], xt[h][cc][:], rstd_b[:])
            x2 = x2p.tile([128, TH], f16, tag="x2")
            nc.vector.tensor_add(x2[:], t1[:], nmr_b[:])
            return x2

        r_out_prev = [None, None]   # AR output dram tiles per half
        LW = {}                     # current layer's weight tiles
        S = {}                      # rolling attention state

        def emit_resm(h):
            for g in range(2):
                rt = rbp.tile([128, 4, TH], f16, tag="rb")
                nc.sync.dma_start(
                    rt[:, :, :],
                    r_out_prev[h][g * 4:(g + 1) * 4, :, :].transpose([1, 0, 2]))
                for k in range(4):
                    cc = g * 4 + k
                    nc.vector.tensor_add(xt[h][cc][:], xt[h][cc][:],
                                         rt[:, k, :])

        def emit_ln(h):
            rstd_b, nmr_b = ln_stats(h)
            return [ln_norm(h, rstd_b, nmr_b, cc) for cc in range(NCH)]

        def emit_qkv(l, h, x2):
            wq_t = LW["wq"]
            qk = []
            for oc in range(4):
                p = psA.tile([128, TH], f32, tag="pmm")
                for cc in range(NCH):
                    nc.tensor.matmul(p[:], wq_t[cc][:, oc * 128:(oc + 1) * 128],
                                     x2[cc][:],
                                     start=(cc == 0), stop=(cc == NCH - 1))
                dst = qkp.tile([128, TH], f16, tag=("qh" if oc < 2 else "kh"))
                nc.vector.tensor_scalar_add(
                    dst[:], p[:], bqk_c[:, l * 4 + oc:l * 4 + oc + 1])
                qk.append(dst)
            vts = []
            for tv in range(4):
                pv = psA.tile([128, 4, HD], f32, tag="pmm")
                for cc in range(NCH):
                    nc.tensor.matmul(pv[:, :, :],
                                     x2[cc][:, tv * 128:(tv + 1) * 128],
                                     wq_t[cc][:, 2 * QO:3 * QO],
                                     start=(cc == 0), stop=(cc == NCH - 1))
                vt = vp.tile([128, 4, HD + 1], bf16, tag="v")
                nc.vector.memset(vt[:, :, HD:HD + 1], 1.0)
                nc.vector.tensor_copy(vt[:, :, 0:HD], pv[:, :, :])
                vts.append(vt)
            S[("q", h)] = qk[:2]
            S[("k", h)] = qk[2:]
            S[("v", h)] = vts

        def emit_scores_head(h, hh):
            qi, ro = hh // 2, (hh % 2) * 64
            for si in range(4 * h + 4):
                c0 = max(h * TH, si * 128)
                w = h * TH + TH - c0
                pool, ptag = ((psB, "pa") if si % 2 else (psA, "pmm"))
                pa = pool.tile([128, TH], f32, tag=ptag)
                nc.tensor.matmul(
                    pa[:, 0:w],
                    S[("k", si // 4)][qi][ro:ro + 64,
                                          (si % 4) * 128:(si % 4) * 128 + 128],
                    S[("q", h)][qi][ro:ro + 64, c0 - h * TH:c0 - h * TH + w],
                    start=True, stop=True)
                abt = abp.tile([128, TH], bf16, tag="ab")
                nc.scalar.activation(abt[:, 0:w], pa[:, 0:w], AF.Exp,
                                     scale=float(SCALE))
                if c0 == si * 128:
                    nc.vector.tensor_mul(abt[:, 0:128], abt[:, 0:128], mask[:])
                S[("ab", h, hh, si)] = (abt, c0)

        def emit_av_head(l, h, hh, ybf):
            py = psC.tile([HD + 1, TH], f32, tag="py")
            for tcl in range(4):
                tcg = 4 * h + tcl
                for si in range(tcg + 1):
                    abt, c0 = S[("ab", h, hh, si)]
                    nc.tensor.matmul(
                        py[:, tcl * 128:(tcl + 1) * 128],
                        S[("v", si // 4)][si % 4][:, hh:hh + 1, :],
                        abt[:, tcg * 128 - c0:tcg * 128 - c0 + 128],
                        start=(si == 0), stop=(si == tcg))
            den = smp.tile([1, TH], f32, tag="den", bufs=3)
            nc.vector.reciprocal(den[:], py[HD:HD + 1, :])
            den_b = dbp.tile([64, TH], f32, tag="db")
            nc.gpsimd.partition_broadcast(den_b[:], den[:])
            yraw = yrp.tile([64, TH], f16, tag="yr")
            nc.vector.tensor_mul(yraw[:], py[0:HD, :], den_b[:])
            i = hh // 2
            if ybf[i] is None:
                ybf[i] = ybp.tile([128, TH], f16, tag="yb",
                                  name=f"yb{l}_{h}_{i}")
            ro2 = (hh % 2) * 64
            nc.scalar.activation(ybf[i][ro2:ro2 + 64, :], yraw[:], AF.Identity,
                                 bias=by_c[ro2:ro2 + 64, l * 2 + i:l * 2 + i + 1])

        def emit_ag(h, ybf):
            g_in = dr.tile([QO, TH], f16, tag=f"gi{h}")
            for i in range(2):
                nc.sync.dma_start(g_in[i * 128:(i + 1) * 128, :], ybf[i][:])
            g_out = dr.tile([NCH, 128, TH], f16, tag=f"go{h}")
            if collectives is True:
                nc.gpsimd.collective_compute(
                    "AllGather", OP.bypass, replica_groups=GROUPS,
                    ins=[g_in.opt()], outs=[g_out.opt()])
            else:
                for q in range(TP):
                    nc.sync.dma_start(g_out[q * 2:(q + 1) * 2, :, :],
                                      g_in[:])
            S[("go", h)] = g_out

        def emit_resy(h):
            for g in range(2):
                yt = rbp.tile([128, 4, TH], f16, tag="rb")
                nc.sync.dma_start(
                    yt[:, :, :],
                    S[("go", h)][g * 4:(g + 1) * 4, :, :].transpose([1, 0, 2]))
                for k in range(4):
                    cc = g * 4 + k
                    nc.vector.tensor_add(xt[h][cc][:], xt[h][cc][:],
                                         yt[:, k, :])

        def emit_mlp1(l, h, x2):
            a_t = []
            for fc in range(NCH):
                pm = psA.tile([128, TH], f32, tag="pmm")
                for cc in range(NCH):
                    nc.tensor.matmul(pm[:],
                                     LW["w1"][cc][:, fc * 128:(fc + 1) * 128],
                                     x2[cc][:],
                                     start=(cc == 0), stop=(cc == NCH - 1))
                ga = ap_.tile([128, TH], f16, tag="a")
                nc.scalar.activation(ga[:], pm[:], AF.Gelu,
                                     bias=b1_c[:, l * 8 + fc:l * 8 + fc + 1])
                a_t.append(ga)
            return a_t

        def emit_mlp2_ar(l, h, a_t):
            r_in = dr.tile([NCH, 128, TH], f16, tag=f"ri{h}")
            for cc in range(NCH):
                pm2 = psA.tile([128, TH], f32, tag="pmm")
                for fc in range(NCH):
                    nc.tensor.matmul(pm2[:],
                                     LW["w2"][fc][:, cc * 128:(cc + 1) * 128],
                                     a_t[fc][:],
                                     start=(fc == 0), stop=(fc == NCH - 1))
                mo = mop.tile([128, TH], f16, tag="mo")
                nc.scalar.activation(mo[:], pm2[:], AF.Identity,
                                     bias=b2_c[:, l * 8 + cc:l * 8 + cc + 1])
                nc.sync.dma_start(r_in[cc, :, :], mo[:])
            r_out = dr.tile([NCH, 128, TH], f16, tag=f"ro{h}")
            if collectives is True:
                nc.gpsimd.collective_compute(
                    "AllReduce", OP.add, replica_groups=GROUPS,
                    ins=[r_in.opt()], outs=[r_out.opt()])
            else:
                for g in range(2):
                    nc.sync.dma_start(r_out[g * 4:(g + 1) * 4, :, :],
                                      r_in[g * 4:(g + 1) * 4, :, :])
            r_out_prev[h] = r_out

        def emit_fin(h):
            rstd_b, nmr_b = ln_stats(h)
            out = []
            for cc in range(NCH):
                t1 = t1p.tile([128, TH], f16, tag="t1")
                nc.vector.tensor_mul(t1[:], xt[h][cc][:], rstd_b[:])
                t2 = t1p.tile([128, TH], f16, tag="t1b")
                nc.vector.tensor_add(t2[:], t1[:], nmr_b[:])
                x2 = x2p.tile([128, TH], f16, tag="x2")
                nc.scalar.activation(x2[:], t2[:], AF.Identity,
                                     bias=bf_c[:, cc:cc + 1])
                out.append(x2)
            return out

        hf = [None, None]
        mark("L0:ln1A")
        x2_next_A = emit_ln(0)      # layer-0 LN1(A) straight from x0
        for l in range(L):
            mark(f"L{l}:w")
            LW["wq"] = []
            for cc in range(NCH):
                t = wqp.tile([128, 3 * QO], f16, tag="wq")
                nc.sync.dma_start(t[:], wqkv_d[l, cc * 128:(cc + 1) * 128, :])
                LW["wq"].append(t)
            LW["w1"], LW["w2"] = [], []
            for cc in range(NCH):
                t = w1p.tile([128, FL], f16, tag="w1")
                nc.sync.dma_start(t[:], w1_d[l, cc * 128:(cc + 1) * 128, :])
                LW["w1"].append(t)
                t = w2p.tile([128, C], f16, tag="w2")
                nc.sync.dma_start(t[:], w2_d[l, cc * 128:(cc + 1) * 128, :])
                LW["w2"].append(t)

            mark(f"L{l}:qkvA")
            emit_qkv(l, 0, x2_next_A)
            mark(f"L{l}:scoA")
            ybfA = [None, None]
            for hh in range(HL):
                emit_scores_head(0, hh)
            mark(f"L{l}:ln1B")
            if l > 0:
                emit_resm(1)
            x2B = emit_ln(1)
            mark(f"L{l}:avA")
            emit_av_head(l, 0, 0, ybfA)
            emit_av_head(l, 0, 1, ybfA)
            emit_av_head(l, 0, 2, ybfA)
            mark(f"L{l}:qkvB")
            emit_qkv(l, 1, x2B)
            mark(f"L{l}:avA2")
            emit_av_head(l, 0, 3, ybfA)
            emit_ag(0, ybfA)
            mark(f"L{l}:attB01")
            ybfB = [None, None]
            emit_scores_head(1, 0)
            emit_scores_head(1, 1)
            emit_av_head(l, 1, 0, ybfB)
            emit_scores_head(1, 2)
            emit_av_head(l, 1, 1, ybfB)
            emit_scores_head(1, 3)
            mark(f"L{l}:ln2A")
            emit_resy(0)
            x2mA = emit_ln(0)
            mark(f"L{l}:attB23")
            emit_av_head(l, 1, 2, ybfB)
            emit_av_head(l, 1, 3, ybfB)
            emit_ag(1, ybfB)
            mark(f"L{l}:mlp1A")
            a_A = emit_mlp1(l, 0, x2mA)
            mark(f"L{l}:mlp2A")
            emit_mlp2_ar(l, 0, a_A)
            mark(f"L{l}:ln2B")
            emit_resy(1)
            x2mB = emit_ln(1)
            mark(f"L{l}:mlp1B")
            a_B = emit_mlp1(l, 1, x2mB)
            mark(f"L{l}:preA")
            emit_resm(0)            # AR(A) of this layer just landed
            if l < L - 1:
                x2_next_A = emit_ln(0)
            else:
                hf[0] = emit_fin(0)
                for vb, (pool, tg) in ((0, (hwp, "hw")), (1, (w1p, "w1"))):
                    for cc in range(NCH):
                        wt = pool.tile([128, 512], f16, tag=tg)
                        nc.sync.dma_start(wt[:, 0:512],
                                          hw_d[cc * 128:(cc + 1) * 128,
                                               vb * 512:vb * 512 + 512])
                        S[("hw", vb, cc)] = wt
            mark(f"L{l}:mlp2B")
            emit_mlp2_ar(l, 1, a_B)

        mark("finB")
        emit_resm(1)
        hf[1] = emit_fin(1)

        NVB = (VL + 511) // 512
        HWPOOLS = [(hwp, "hw"), (w1p, "w1"), (w2p, "w2")]

        def load_hw(vb):
            vn = min(512, VL - vb * 512)
            pool, tg = HWPOOLS[vb % 3]
            ts = []
            for cc in range(NCH):
                wt = pool.tile([128, 512], f16, tag=tg)
                nc.sync.dma_start(wt[:, 0:vn],
                                  hw_d[cc * 128:(cc + 1) * 128,
                                       vb * 512:vb * 512 + vn])
                ts.append(wt)
            return ts

        pend = [[S[("hw", vb, cc)] for cc in range(NCH)] for vb in range(2)]
        pend.append(load_hw(2))
        pend.append(load_hw(3))
        PSH = [psA, psB, psC]
        for vb in range(NVB):
            vn = min(512, VL - vb * 512)
            rhs_t = pend.pop(0)
            if vb + 4 < NVB:
                pend.append(load_hw(vb + 4))
            for tcc in range(8):
                h, tl = tcc // 4, tcc % 4
                pool = PSH[tcc % 3]
                ph = pool.tile([128, 512], f32,
                               tag={0: "pmm", 1: "pa", 2: "py"}[tcc % 3])
                for cc in range(NCH):
                    nc.tensor.matmul(ph[:, 0:vn],
                                     hf[h][cc][:, tl * 128:(tl + 1) * 128],
                                     rhs_t[cc][:, 0:vn],
                                     start=(cc == 0), stop=(cc == NCH - 1))
                so = outp.tile([128, 512], f16, tag="so")
                if tcc % 2:
                    nc.vector.tensor_copy(so[:, 0:vn], ph[:, 0:vn])
                else:
                    nc.scalar.activation(so[:, 0:vn], ph[:, 0:vn], AF.Copy)
                nc.sync.dma_start(out_d[tcc * 128:(tcc + 1) * 128,
                                        vb * 512:vb * 512 + vn],
                                  so[:, 0:vn])
            mark(f"head{vb}")


def _prep_inputs(idx, tok_emb, pos_emb, ln1_w, ln1_b, wq, bq, wk, bk, wv, bv,
                 ln2_w, ln2_b, w1, b1, w2, b2, lnf_w, lnf_b, head_w):
    fh = np.float16

    mask = np.zeros((128, 128), np.float32)
    p, t = np.meshgrid(np.arange(128), np.arange(128), indexing="ij")
    mask[p <= t] = 1.0
    mask = mask.astype(ml_dtypes.bfloat16)

    x0s = []
    for g in range(B):
        x0 = tok_emb[np.asarray(idx[g], np.int64)] + pos_emb[0]
        x0s.append(np.ascontiguousarray(x0.T, np.float32).astype(fh))

    ln1w = np.asarray(ln1_w, np.float32)
    ln1b = np.asarray(ln1_b, np.float32)
    ln2w = np.asarray(ln2_w, np.float32)
    ln2b = np.asarray(ln2_b, np.float32)

    in_maps = []
    shard_cache = {}
    for c in range(8):
        g, j = c // 4, c % 4
        if j in shard_cache:
            m = dict(shard_cache[j])
            m["x0t"] = x0s[g]
            in_maps.append(m)
            continue
        sl = slice(j * QO, (j + 1) * QO)
        wq_j = np.asarray(wq[:, :, sl], np.float32)
        wk_j = np.asarray(wk[:, :, sl], np.float32)
        wv_j = np.asarray(wv[:, :, sl], np.float32)
        w1_j = np.asarray(w1[:, :, j * FL:(j + 1) * FL], np.float32)
        w2_j = np.asarray(w2[:, j * FL:(j + 1) * FL, :], np.float32)
        hw_j = np.asarray(head_w[:, j * VL:(j + 1) * VL], np.float32)

        # fold LN scale into the consuming weights
        wqkv = np.concatenate([wq_j, wk_j, wv_j], axis=2) * ln1w[:, :, None]
        w1f = w1_j * ln2w[:, :, None]
        hwf = hw_j * np.asarray(lnf_w, np.float32)[:, None]

        # bias columns: layer bias + W^T ln_b (the folded-LN bias term)
        # bqk: [128, L*4], col l*4 + oc covers q0,q1,k0,k1
        bqk = np.zeros((L, 4, 128), np.float32)
        byc = np.zeros((L, 2, 128), np.float32)
        for l in range(L):
            q_eff = bq[l, sl] + wq_j[l].T @ ln1b[l]
            k_eff = bk[l, sl] + wk_j[l].T @ ln1b[l]
            v_eff = bv[l, sl] + wv_j[l].T @ ln1b[l]
            bqk[l, 0] = q_eff[0:128]
            bqk[l, 1] = q_eff[128:256]
            bqk[l, 2] = k_eff[0:128]
            bqk[l, 3] = k_eff[128:256]
            byc[l, 0] = v_eff[0:128]
            byc[l, 1] = v_eff[128:256]
        bqk = np.ascontiguousarray(bqk.reshape(L * 4, 128).T)
        byc = np.ascontiguousarray(byc.reshape(L * 2, 128).T)

        b1c = np.zeros((L, NCH, 128), np.float32)
        for l in range(L):
            f_eff = b1[l, j * FL:(j + 1) * FL] + w1_j[l].T @ ln2b[l]
            b1c[l] = f_eff.reshape(NCH, 128)
        b1c = np.ascontiguousarray(b1c.reshape(L * 8, 128).T)

        b2c = np.ascontiguousarray(
            (np.asarray(b2, np.float32) / TP).reshape(L, NCH, 128)
            .reshape(L * 8, 128).T)

        lw = np.asarray(lnf_w, np.float32)
        lb = np.asarray(lnf_b, np.float32)
        bfc = np.where(np.abs(lw) > 1e-12, lb / np.where(lw == 0, 1, lw), 0.0)
        bfc = np.ascontiguousarray(bfc.reshape(NCH, 128).T.astype(np.float32))

        m = {
            "x0t": x0s[g],
            "wqkv": np.ascontiguousarray(wqkv).astype(fh),
            "w1": np.ascontiguousarray(w1f).astype(fh),
            "w2": np.ascontiguousarray(w2_j).astype(fh),
            "hw": np.ascontiguousarray(hwf).astype(fh),
            "bqk": bqk, "byc": byc, "b1c": b1c, "b2c": b2c, "bfc": bfc,
            "mask": mask,
        }
        shard_cache[j] = m
        in_maps.append(m)
    return in_maps


def kernel(**inputs):
    if "nc" not in _STATE:
        _STATE["nc"] = _build()
    nc = _STATE["nc"]
    in_maps = _prep_inputs(**{k: np.asarray(v) for k, v in inputs.items()})
    res = bass_utils.run_bass_kernel_spmd(nc, in_maps, core_ids=list(range(8)))
    outs = res.results
    full = np.empty((B, T, V), np.float32)
    for c in range(8):
        g, j = c // 4, c % 4
        full[g, :, j * VL:(j + 1) * VL] = np.asarray(outs[c]["out"],
                                                     np.float32)
    return full
